# revision 51
# baseline (speedup 1.0000x reference)
"""Multi-head attention (B=2,T=2048,C=1024,H=16,RoPE,causal) on 8 TRN2 cores.

Sharding: core c -> (batch b = c//4, head-group g = c%4, heads [4g,4g+4)).
Each core computes QKV projection for its 4 heads against x[b], RoPE,
causal attention in transposed-score layout [s, t], and the output
projection rows t' in [512g, 512g+512) of y[b] (the reference's
(B,H,T,Dh)->(B,T,C) reshape makes output blocks head-disjoint).

Schedule: stage A (proj+RoPE+vT) and stage B (attention) interleaved at
t-tile granularity (A0 A1 B0 A2 B1 A3 B2 B3) with a software-pipelined
QK->exp->AV chunk loop (QK[j+1] issued before AV[j]) so the PE never
waits on the activation engine.  All PSUM->SBUF staging copies run on
ACT/DVE/Pool chosen to balance engine load; exp instructions cover both
heads of a pair via strided APs.
"""
import math
import sys

sys.path.insert(0, '/opt/trn_rl_repo')
sys.path.insert(0, '/opt/pypackages')

import ml_dtypes
import numpy as np
from contextlib import ExitStack

import concourse.bass as bass  # noqa: F401
import concourse.tile as tile
from concourse import bacc, mybir
from concourse.bass_utils import run_bass_kernel_spmd

BF16 = mybir.dt.bfloat16
F32 = mybir.dt.float32
NPBF16 = ml_dtypes.bfloat16
EXP = mybir.ActivationFunctionType.Exp

B, T, C, H, Dh = 2, 2048, 1024, 16, 64
HALF = Dh // 2          # 32
NCORES = 8
HPC = 4                 # heads per core
CPC = HPC * Dh          # channels per core = 256
SCALE = 1.0 / math.sqrt(Dh)
TT = 512                # t-tile width
NTT = T // TT           # 4
SC = 128                # s-chunk width

_compiled_nc = None


def _calibrate_cost_model():
    """Calibrate the tile scheduler's cost model to measured HW speeds so
    its static schedule interleaves enough work to cover real ACT/DVE
    latencies (the stock model is optimistic and the fixed instruction
    order then stalls on hardware)."""
    from concourse import hw_specs
    spec = hw_specs.TRN2Spec
    if getattr(spec, "_mha_calibrated", False):
        return
    spec._mha_calibrated = True
    spec.CYCLE_T = {**spec.CYCLE_T,
                    mybir.EngineType.Activation: 1e9 / 0.90e9,
                    mybir.EngineType.DVE: 1e9 / 0.85e9}
    spec.SEM_DELAY = 120
    spec.PE_CYCLE = 1e9 / 2.07e9
    spec.DMA_CYCLE = spec.DMA_CYCLE * 1.5


def _build_nc(dbg=False):
    _calibrate_cost_model()
    nc = bacc.Bacc("TRN2", target_bir_lowering=False, debug=False)

    xr = nc.dram_tensor("xr", [NTT, 128, 8, TT], BF16, kind="ExternalInput").ap()
    wqkvr = nc.dram_tensor("wqkvr", [3, 128, 8, CPC], BF16,
                           kind="ExternalInput").ap()
    wtr = nc.dram_tensor("wtr", [128, 8, C], BF16, kind="ExternalInput").ap()
    cosx = nc.dram_tensor("cosx", [128, T], BF16, kind="ExternalInput").ap()
    sinx = nc.dram_tensor("sinx", [128, T], BF16, kind="ExternalInput").ap()
    rt = nc.dram_tensor("rt", [128, 128], BF16, kind="ExternalInput").ap()
    idb = nc.dram_tensor("idb", [128, 128], BF16, kind="ExternalInput").ap()
    mask2 = nc.dram_tensor("mask2", [128, 2, 128], BF16,
                           kind="ExternalInput").ap()
    yblk = nc.dram_tensor("yblk", [4, 2, 128, 512], F32, kind="ExternalOutput").ap()
    if dbg:
        qdbg = nc.dram_tensor("qdbg", [128, T], BF16, kind="ExternalOutput").ap()
        kdbg = nc.dram_tensor("kdbg", [128, T], BF16, kind="ExternalOutput").ap()
        vdbg = nc.dram_tensor("vdbg", [128, 16, Dh + 1], BF16,
                              kind="ExternalOutput").ap()
        adbg = nc.dram_tensor("adbg", [128, 4, TT], BF16,
                              kind="ExternalOutput").ap()
        hdbg = nc.dram_tensor("hdbg", [128, 8, 128], BF16,
                              kind="ExternalOutput").ap()

    with tile.TileContext(nc) as tc, ExitStack() as ctx:
        const = ctx.enter_context(tc.tile_pool(name="const", bufs=1))
        qkpool = ctx.enter_context(tc.tile_pool(name="qk", bufs=2))
        vpool = ctx.enter_context(tc.tile_pool(name="vnat", bufs=4))
        attp = ctx.enter_context(tc.tile_pool(name="attp", bufs=2))
        tmp = ctx.enter_context(tc.tile_pool(name="tmp", bufs=2))
        pbp = ctx.enter_context(tc.tile_pool(name="pbp", bufs=4))
        ahpool = ctx.enter_context(tc.tile_pool(name="ahp", bufs=2))
        psMM = ctx.enter_context(tc.tile_pool(name="psMM", bufs=2, space="PSUM"))
        psQK = ctx.enter_context(tc.tile_pool(name="psQK", bufs=2, space="PSUM"))
        psAC = ctx.enter_context(tc.tile_pool(name="psAC", bufs=2, space="PSUM"))

        # ---- input loads, in order of first use ----
        wqkv_sb = const.tile([128, 3, 8, CPC], BF16)
        nc.sync.dma_start(wqkv_sb[:, 0], wqkvr[0])
        x_sb = []
        for tt in range(NTT):
            x_sb.append(const.tile([128, 8, TT], BF16, name=f"x_sb{tt}"))
        nc.sync.dma_start(x_sb[0][:], xr[0])
        nc.sync.dma_start(wqkv_sb[:, 1], wqkvr[1])
        nc.sync.dma_start(wqkv_sb[:, 2], wqkvr[2])
        rt_sb = const.tile([128, 128], BF16)
        nc.sync.dma_start(rt_sb[:], rt[:])
        id_sb = const.tile([128, 128], BF16)
        nc.sync.dma_start(id_sb[:], idb[:])
        cos_sb = const.tile([128, T], BF16)
        nc.sync.dma_start(cos_sb[:], cosx[:])
        sin_sb = const.tile([128, T], BF16)
        nc.sync.dma_start(sin_sb[:], sinx[:])
        nc.sync.dma_start(x_sb[1][:], xr[1])
        mask_sb = const.tile([128, 2, 128], BF16)
        nc.sync.dma_start(mask_sb[:], mask2[:])
        nc.sync.dma_start(x_sb[2][:], xr[2])
        nc.sync.dma_start(x_sb[3][:], xr[3])
        wt_sb = const.tile([128, 8, C], BF16)
        nc.sync.dma_start(wt_sb[:], wtr[:])

        def stage_a(hp, tt, q_sb, k_sb, v_nat):
            """QKV projection + RoPE + V transpose for one t-tile."""
            ts = slice(tt * TT, (tt + 1) * TT)
            gps = {}
            gb = {}
            rot = {}
            # q/k projections, with RoPE staged behind each
            for grp, gi in (("q", 0), ("k", 1)):
                f0 = hp * 128
                gps[grp] = psMM.tile([128, TT], F32, tag="mm",
                                     name=f"gps_{grp}_{hp}_{tt}")
                for cc in range(8):
                    nc.tensor.matmul(gps[grp][:],
                                     wqkv_sb[:, gi, cc, f0:f0 + 128],
                                     x_sb[tt][:, cc, :],
                                     start=(cc == 0), stop=(cc == 7))
                # PSUM -> SBUF bf16 staging copy on ACT
                gb[grp] = tmp.tile([128, TT], BF16, tag=f"gb{grp}", name=f"gb_{grp}_{hp}_{tt}")
                with tc.high_priority():
                    nc.any.tensor_copy(gb[grp][:], gps[grp][:])
                if grp == "k":
                    # rot-q emitted here so PE has work while gb-k copies
                    rot["q"] = psMM.tile([128, TT], F32, tag="mm",
                                         name=f"rot_q_{hp}_{tt}")
                    nc.tensor.matmul(rot["q"][:], rt_sb[:], gb["q"][:],
                                     start=True, stop=True)
            # RoPE combine for q: dest = gb*cos (DVE 2x) + rot*sin (Pool)
            def rope_combine(grp, dest):
                m1 = tmp.tile([128, TT], BF16, tag="m1", name=f"m1_{grp}_{hp}_{tt}")
                m2 = tmp.tile([128, TT], BF16, tag="m2", name=f"m2_{grp}_{hp}_{tt}")
                with tc.high_priority():
                    nc.gpsimd.tensor_mul(m1[:], gb[grp][:], cos_sb[:, ts])
                    nc.vector.tensor_mul(m2[:], rot[grp][:], sin_sb[:, ts])
                    nc.vector.tensor_add(dest[:, ts], m1[:], m2[:])

            rope_combine("q", q_sb)
            # v projection (PE work covering the q-combine + rot-q release)
            f0 = hp * 128
            gps["v"] = psMM.tile([128, TT], F32, tag="mm",
                                 name=f"gps_v_{hp}_{tt}")
            for cc in range(8):
                nc.tensor.matmul(gps["v"][:], wqkv_sb[:, 2, cc, f0:f0 + 128],
                                 x_sb[tt][:, cc, :],
                                 start=(cc == 0), stop=(cc == 7))
            vf = tmp.tile([128, TT], BF16, tag="vf", name=f"vf_{hp}_{tt}")
            with tc.high_priority():
                nc.vector.tensor_copy(vf[:], gps["v"][:])
            rot["k"] = psMM.tile([128, TT], F32, tag="mm",
                                 name=f"rot_k_{hp}_{tt}")
            nc.tensor.matmul(rot["k"][:], rt_sb[:], gb["k"][:],
                             start=True, stop=True)
            rope_combine("k", k_sb)
            # V transpose: 4x [128,128] bf16 PE transposes into one PSUM tile
            tps = psMM.tile([128, 4, 128], BF16, tag="mm",
                            name=f"tps_{hp}_{tt}")
            for st in range(4):
                nc.tensor.transpose(tps[:, st, :], vf[:, st * 128:(st + 1) * 128],
                                    id_sb[:])
            with tc.high_priority():
                for hl in range(2):
                    nc.vector.tensor_copy(
                        v_nat[hl][:, 4 * tt:4 * tt + 4, 0:Dh],
                        tps[:, :, hl * 64:hl * 64 + 64])

        def stage_b(hp, tt, q_sb, k_sb, v_nat, att_sb):
            """Causal attention for queries in tile tt, pipelined chunks.

            The QK moving operand reads q in (j,k')-major order (tau = j*32+k',
            t = 512*tt + 16*k' + j), free for the PE, so every downstream
            elementwise op is contiguous."""
            ts = slice(tt * TT, (tt + 1) * TT)
            njs = 4 * tt + 4
            acc = [psAC.tile([Dh + 1, TT], F32, tag="acc",
                             name=f"acc_{hp}_{tt}_{hl}") for hl in range(2)]
            pend = []  # (j, pb) entries awaiting AV, lag 2 behind QK
            for j in range(njs):
                sj = slice(j * SC, (j + 1) * SC)
                qk = psQK.tile([128, 2, TT], F32, tag="qk",
                               name=f"qk_{hp}_{tt}_{j}")
                for hl in range(2):
                    hb = hl * 64
                    nc.tensor.matmul(qk[:, hl], k_sb[hb:hb + 64, sj],
                                     q_sb[hb:hb + 64, ts],
                                     start=True, stop=True)
                pb = pbp.tile([128, 2, TT], BF16, tag="pb",
                              name=f"pb_{hp}_{tt}_{j}")
                c = j - 4 * tt
                if c < 0:
                    nc.scalar.activation(pb[:], qk[:], EXP, scale=SCALE)
                else:
                    off = 128 * c
                    if c == 0:
                        nc.scalar.activation(pb[:], qk[:], EXP, scale=SCALE)
                    else:
                        nc.gpsimd.memset(pb[:, :, 0:off], 0.0)
                        nc.scalar.activation(pb[:, :, off:], qk[:, :, off:],
                                             EXP, scale=SCALE)
                    nc.vector.tensor_mul(pb[:, :, off:off + 128],
                                         pb[:, :, off:off + 128],
                                         mask_sb[:])
                if len(pend) >= 2:
                    pj, ppb = pend.pop(0)
                    for hl in range(2):
                        nc.tensor.matmul(acc[hl][:], v_nat[hl][:, pj, :],
                                         ppb[:, hl],
                                         start=(pj == 0), stop=False)
                pend.append((j, pb))
            for pj, ppb in pend:
                for hl in range(2):
                    nc.tensor.matmul(acc[hl][:], v_nat[hl][:, pj, :],
                                     ppb[:, hl],
                                     start=(pj == 0), stop=(pj == njs - 1))
            # normalize into att_sb (layout [d, tt, tau])
            for hl in range(2):
                zrow = tmp.tile([1, TT], F32, tag="zrow",
                                name=f"zrow_{hp}_{tt}_{hl}")
                zi = tmp.tile([1, TT], F32, tag="zi", name=f"zi_{hp}_{tt}_{hl}")
                zb = tmp.tile([64, TT], F32, tag="zb", name=f"zb_{hp}_{tt}_{hl}")
                with tc.high_priority(offset=300):
                    nc.vector.tensor_copy(zrow[:], acc[hl][Dh:Dh + 1, :])
                    nc.vector.reciprocal_approx_fast(out=zi[:], in_=zrow[:])
                    nc.gpsimd.partition_broadcast(zb[:], zi[:], channels=64)
                    nc.vector.tensor_mul(att_sb[hl * 64:hl * 64 + 64, tt, :],
                                         acc[hl][0:Dh, :], zb[:])

        def repack(hp, att_sb, ahts):
            """att [d, tt, (k' j)] (t-linear) -> aht [two*64+d, cc, (tt k')]."""
            for hl in range(2):
                aht = ahpool.tile([128, 8, 128], BF16, tag="aht",
                                  name=f"aht_{hp}_{hl}")
                attv = att_sb[hl * 64:hl * 64 + 64].rearrange(
                    "d tt (k j) -> d j tt k", j=16)
                for two in range(2):
                    for cc in range(8):
                        nc.any.tensor_copy(
                            aht[two * 64:two * 64 + 64, cc, :].rearrange(
                                "d (tt k) -> d tt k", tt=4),
                            attv[:, 2 * cc + two])
                ahts.append(aht)

        def yproj(hp, hl, aht):
            blk = hp * 2 + hl
            for ot in range(2):
                ypss = psMM.tile([128, 512], F32, tag="mm",
                                 name=f"yps_{hp}_{hl}_{ot}")
                for cc in range(8):
                    nc.tensor.matmul(ypss[:], aht[:, cc, :],
                                     wt_sb[:, cc, ot * 512:(ot + 1) * 512],
                                     start=(cc == 0), stop=(cc == 7))
                yo = tmp.tile([128, 512], F32, tag="yo", bufs=4,
                              name=f"yo_{hp}_{hl}_{ot}")
                with tc.high_priority():
                    nc.any.tensor_copy(yo[:], ypss[:])
                nc.sync.dma_start(yblk[blk, ot], yo[:])

        prev = None  # (att_sb, ahts, hp) of previous head pair
        for hp in range(2):
            q_sb = qkpool.tile([128, T], BF16, tag="q", name=f"q_{hp}")
            k_sb = qkpool.tile([128, T], BF16, tag="k", name=f"k_{hp}")
            v_nat = [vpool.tile([128, T // SC, Dh + 1], BF16, tag="vnat",
                                name=f"vnat_{hp}_{hl}") for hl in range(2)]
            for hl in range(2):
                nc.gpsimd.memset(v_nat[hl][:, :, Dh:Dh + 1], 1.0)
            att_sb = attp.tile([128, 4, TT], BF16, tag="att",
                               name=f"att_{hp}")

            for tt in range(NTT):
                stage_a(hp, tt, q_sb, k_sb, v_nat)
            if prev is not None:
                # output projection of previous head pair; the scheduler
                # slots these into this head pair's ACT-bound windows
                patt, pahts, php = prev
                yproj(php, 0, pahts[0])
                yproj(php, 1, pahts[1])
                prev = None
            ahts = []
            for tt in range(NTT):
                stage_b(hp, tt, q_sb, k_sb, v_nat, att_sb)
            repack(hp, att_sb, ahts)
            prev = (att_sb, ahts, hp)
            if dbg and hp == 0:
                nc.sync.dma_start(qdbg[:], q_sb[:])
                nc.sync.dma_start(kdbg[:], k_sb[:])
                nc.sync.dma_start(vdbg[:], v_nat[0][:])
                nc.sync.dma_start(adbg[:], att_sb[:])
                nc.sync.dma_start(hdbg[:], ahts[0][:])

        patt, pahts, php = prev
        yproj(php, 0, pahts[0])
        yproj(php, 1, pahts[1])

    nc.compile()
    return nc


def _get_nc():
    global _compiled_nc
    if _compiled_nc is None:
        _compiled_nc = _build_nc()
    return _compiled_nc


def _host_tables():
    pos = np.arange(T, dtype=np.float32)[:, None]
    inv = np.exp(np.arange(0, Dh, 2, dtype=np.float32)
                 * (-math.log(10000.0) / Dh))
    ang = pos * inv                       # (T, 32)
    sin, cos = np.sin(ang), np.cos(ang)   # (T, 32)
    idx = np.arange(128) % HALF           # d % 32
    cos_ext = np.ascontiguousarray(cos[:, idx].T).astype(NPBF16)  # (128, T)
    sin_ext = np.ascontiguousarray(sin[:, idx].T).astype(NPBF16)

    R = np.zeros((128, 128), dtype=np.float32)
    for blk in (0, 64):
        for m in range(HALF):
            R[blk + m, blk + m + HALF] = -1.0
            R[blk + m + HALF, blk + m] = 1.0
    rt = np.ascontiguousarray(R.T).astype(NPBF16)

    s_i = np.arange(128)[:, None]
    t_i = np.arange(128)[None, :]
    mask01 = (t_i >= s_i).astype(np.float32).astype(NPBF16)
    mask2 = np.ascontiguousarray(
        np.broadcast_to(mask01[:, None, :], (128, 2, 128)))
    ident = np.eye(128, dtype=np.float32).astype(NPBF16)
    return cos_ext, sin_ext, rt, mask2, ident


def kernel(x, w_qkv, w_proj):
    x = np.asarray(x)
    w_qkv = np.asarray(w_qkv)
    w_proj = np.asarray(w_proj)
    nc = _get_nc()
    in_maps = build_in_maps(x, w_qkv, w_proj)
    res = run_bass_kernel_spmd(nc, in_maps, core_ids=list(range(NCORES)))
    y = np.zeros((B, T, C), dtype=np.float32)
    for c in range(NCORES):
        b, g = c // 4, c % 4
        yb = res.results[c]["yblk"]  # [4, 2, 128, 512]
        y[b, 512 * g:512 * g + 512, :] = yb.transpose(0, 2, 1, 3).reshape(512, C)
    return y


def build_in_maps(x, w_qkv, w_proj):
    cos_ext, sin_ext, rt, mask2, ident = _host_tables()
    wq4 = w_qkv.reshape(3, H, Dh, C)
    # w_proj^T packed [p, cc, o]
    wtr = np.ascontiguousarray(
        w_proj.T.reshape(8, 128, C).transpose(1, 0, 2)).astype(NPBF16)
    in_maps = []
    xr_cache = {}
    for c in range(NCORES):
        b, g = c // 4, c % 4
        hs = slice(4 * g, 4 * g + 4)
        wq = wq4[0, hs].reshape(CPC, C)
        wk = wq4[1, hs].reshape(CPC, C)
        wv = wq4[2, hs].reshape(CPC, C)
        # [p, g, cc, f]: wqkvr[p, g, cc, f] = w_g[f, cc*128+p] (g in q,k,v)
        wqkvr = np.ascontiguousarray(
            np.stack([wq, wk, wv], 0).reshape(3, CPC, 8, 128)
            .transpose(0, 3, 2, 1)).astype(NPBF16)
        if b not in xr_cache:
            xT = x[b].T  # (C, T)
            xr_cache[b] = np.ascontiguousarray(
                xT.reshape(8, 128, NTT, TT).transpose(2, 1, 0, 3)).astype(NPBF16)
        in_maps.append({
            "xr": xr_cache[b],
            "wqkvr": wqkvr,
            "wtr": wtr,
            "cosx": cos_ext, "sinx": sin_ext,
            "rt": rt, "idb": ident, "mask2": mask2,
        })
    return in_maps


# revision 52
# speedup vs baseline: 1.0225x; 1.0225x over previous
"""Multi-head attention (B=2,T=2048,C=1024,H=16,RoPE,causal) on 8 TRN2 cores.

Sharding: core c -> (batch b = c//4, head-group g = c%4, heads [4g,4g+4)).
Each core computes QKV projection for its 4 heads against x[b], RoPE,
causal attention in transposed-score layout [s, t], and the output
projection rows t' in [512g, 512g+512) of y[b] (the reference's
(B,H,T,Dh)->(B,T,C) reshape makes output blocks head-disjoint).

Schedule: stage A (proj+RoPE+vT) and stage B (attention) interleaved at
t-tile granularity (A0 A1 B0 A2 B1 A3 B2 B3) with a software-pipelined
QK->exp->AV chunk loop (QK[j+1] issued before AV[j]) so the PE never
waits on the activation engine.  All PSUM->SBUF staging copies run on
ACT/DVE/Pool chosen to balance engine load; exp instructions cover both
heads of a pair via strided APs.
"""
import math
import sys

sys.path.insert(0, '/opt/trn_rl_repo')
sys.path.insert(0, '/opt/pypackages')

import ml_dtypes
import numpy as np
from contextlib import ExitStack

import concourse.bass as bass  # noqa: F401
import concourse.tile as tile
from concourse import bacc, mybir
from concourse.bass_utils import run_bass_kernel_spmd

BF16 = mybir.dt.bfloat16
F32 = mybir.dt.float32
NPBF16 = ml_dtypes.bfloat16
EXP = mybir.ActivationFunctionType.Exp

B, T, C, H, Dh = 2, 2048, 1024, 16, 64
HALF = Dh // 2          # 32
NCORES = 8
HPC = 4                 # heads per core
CPC = HPC * Dh          # channels per core = 256
SCALE = 1.0 / math.sqrt(Dh)
TT = 512                # t-tile width
NTT = T // TT           # 4
SC = 128                # s-chunk width

_compiled_nc = None


def _calibrate_cost_model():
    """Calibrate the tile scheduler's cost model to measured HW speeds so
    its static schedule interleaves enough work to cover real ACT/DVE
    latencies (the stock model is optimistic and the fixed instruction
    order then stalls on hardware)."""
    from concourse import hw_specs
    spec = hw_specs.TRN2Spec
    if getattr(spec, "_mha_calibrated", False):
        return
    spec._mha_calibrated = True
    spec.CYCLE_T = {**spec.CYCLE_T,
                    mybir.EngineType.Activation: 1e9 / 0.915e9,
                    mybir.EngineType.DVE: 1e9 / 0.85e9}
    spec.SEM_DELAY = 120
    spec.PE_CYCLE = 1e9 / 2.1e9
    spec.DMA_CYCLE = spec.DMA_CYCLE * 1.5


def _build_nc(dbg=False):
    _calibrate_cost_model()
    nc = bacc.Bacc("TRN2", target_bir_lowering=False, debug=False)

    xr = nc.dram_tensor("xr", [NTT, 128, 8, TT], BF16, kind="ExternalInput").ap()
    wqkvr = nc.dram_tensor("wqkvr", [3, 128, 8, CPC], BF16,
                           kind="ExternalInput").ap()
    wtr = nc.dram_tensor("wtr", [128, 8, C], BF16, kind="ExternalInput").ap()
    cosx = nc.dram_tensor("cosx", [128, T], BF16, kind="ExternalInput").ap()
    sinx = nc.dram_tensor("sinx", [128, T], BF16, kind="ExternalInput").ap()
    rt = nc.dram_tensor("rt", [128, 128], BF16, kind="ExternalInput").ap()
    idb = nc.dram_tensor("idb", [128, 128], BF16, kind="ExternalInput").ap()
    mask2 = nc.dram_tensor("mask2", [128, 2, 128], BF16,
                           kind="ExternalInput").ap()
    yblk = nc.dram_tensor("yblk", [4, 2, 128, 512], F32, kind="ExternalOutput").ap()
    if dbg:
        qdbg = nc.dram_tensor("qdbg", [128, T], BF16, kind="ExternalOutput").ap()
        kdbg = nc.dram_tensor("kdbg", [128, T], BF16, kind="ExternalOutput").ap()
        vdbg = nc.dram_tensor("vdbg", [128, 16, Dh + 1], BF16,
                              kind="ExternalOutput").ap()
        adbg = nc.dram_tensor("adbg", [128, 4, TT], BF16,
                              kind="ExternalOutput").ap()
        hdbg = nc.dram_tensor("hdbg", [128, 8, 128], BF16,
                              kind="ExternalOutput").ap()

    with tile.TileContext(nc) as tc, ExitStack() as ctx:
        const = ctx.enter_context(tc.tile_pool(name="const", bufs=1))
        qkpool = ctx.enter_context(tc.tile_pool(name="qk", bufs=2))
        vpool = ctx.enter_context(tc.tile_pool(name="vnat", bufs=4))
        attp = ctx.enter_context(tc.tile_pool(name="attp", bufs=2))
        tmp = ctx.enter_context(tc.tile_pool(name="tmp", bufs=2))
        pbp = ctx.enter_context(tc.tile_pool(name="pbp", bufs=4))
        ahpool = ctx.enter_context(tc.tile_pool(name="ahp", bufs=2))
        psMM = ctx.enter_context(tc.tile_pool(name="psMM", bufs=2, space="PSUM"))
        psQK = ctx.enter_context(tc.tile_pool(name="psQK", bufs=2, space="PSUM"))
        psAC = ctx.enter_context(tc.tile_pool(name="psAC", bufs=2, space="PSUM"))

        # ---- input loads, in order of first use ----
        wqkv_sb = const.tile([128, 3, 8, CPC], BF16)
        nc.sync.dma_start(wqkv_sb[:, 0], wqkvr[0])
        x_sb = []
        for tt in range(NTT):
            x_sb.append(const.tile([128, 8, TT], BF16, name=f"x_sb{tt}"))
        nc.sync.dma_start(x_sb[0][:], xr[0])
        nc.sync.dma_start(wqkv_sb[:, 1], wqkvr[1])
        nc.sync.dma_start(wqkv_sb[:, 2], wqkvr[2])
        rt_sb = const.tile([128, 128], BF16)
        nc.sync.dma_start(rt_sb[:], rt[:])
        id_sb = const.tile([128, 128], BF16)
        nc.sync.dma_start(id_sb[:], idb[:])
        cos_sb = const.tile([128, T], BF16)
        nc.sync.dma_start(cos_sb[:], cosx[:])
        sin_sb = const.tile([128, T], BF16)
        nc.sync.dma_start(sin_sb[:], sinx[:])
        nc.sync.dma_start(x_sb[1][:], xr[1])
        mask_sb = const.tile([128, 2, 128], BF16)
        nc.sync.dma_start(mask_sb[:], mask2[:])
        nc.sync.dma_start(x_sb[2][:], xr[2])
        nc.sync.dma_start(x_sb[3][:], xr[3])
        wt_sb = const.tile([128, 8, C], BF16)
        nc.sync.dma_start(wt_sb[:], wtr[:])

        def stage_a(hp, tt, q_sb, k_sb, v_nat):
            """QKV projection + RoPE + V transpose for one t-tile."""
            ts = slice(tt * TT, (tt + 1) * TT)
            gps = {}
            gb = {}
            rot = {}
            # q/k projections, with RoPE staged behind each
            for grp, gi in (("q", 0), ("k", 1)):
                f0 = hp * 128
                gps[grp] = psMM.tile([128, TT], F32, tag="mm",
                                     name=f"gps_{grp}_{hp}_{tt}")
                for cc in range(8):
                    nc.tensor.matmul(gps[grp][:],
                                     wqkv_sb[:, gi, cc, f0:f0 + 128],
                                     x_sb[tt][:, cc, :],
                                     start=(cc == 0), stop=(cc == 7))
                # PSUM -> SBUF bf16 staging copy on ACT
                gb[grp] = tmp.tile([128, TT], BF16, tag=f"gb{grp}", name=f"gb_{grp}_{hp}_{tt}")
                with tc.high_priority():
                    nc.any.tensor_copy(gb[grp][:], gps[grp][:])
                if grp == "k":
                    # rot-q emitted here so PE has work while gb-k copies
                    rot["q"] = psMM.tile([128, TT], F32, tag="mm",
                                         name=f"rot_q_{hp}_{tt}")
                    nc.tensor.matmul(rot["q"][:], rt_sb[:], gb["q"][:],
                                     start=True, stop=True)
            # RoPE combine for q: dest = gb*cos (DVE 2x) + rot*sin (Pool)
            def rope_combine(grp, dest):
                m1 = tmp.tile([128, TT], BF16, tag="m1", name=f"m1_{grp}_{hp}_{tt}")
                m2 = tmp.tile([128, TT], BF16, tag="m2", name=f"m2_{grp}_{hp}_{tt}")
                with tc.high_priority():
                    nc.gpsimd.tensor_mul(m1[:], gb[grp][:], cos_sb[:, ts])
                    nc.vector.tensor_mul(m2[:], rot[grp][:], sin_sb[:, ts])
                    nc.vector.tensor_add(dest[:, ts], m1[:], m2[:])

            rope_combine("q", q_sb)
            # v projection (PE work covering the q-combine + rot-q release)
            f0 = hp * 128
            gps["v"] = psMM.tile([128, TT], F32, tag="mm",
                                 name=f"gps_v_{hp}_{tt}")
            for cc in range(8):
                nc.tensor.matmul(gps["v"][:], wqkv_sb[:, 2, cc, f0:f0 + 128],
                                 x_sb[tt][:, cc, :],
                                 start=(cc == 0), stop=(cc == 7))
            vf = tmp.tile([128, TT], BF16, tag="vf", name=f"vf_{hp}_{tt}")
            with tc.high_priority():
                nc.vector.tensor_copy(vf[:], gps["v"][:])
            rot["k"] = psMM.tile([128, TT], F32, tag="mm",
                                 name=f"rot_k_{hp}_{tt}")
            nc.tensor.matmul(rot["k"][:], rt_sb[:], gb["k"][:],
                             start=True, stop=True)
            rope_combine("k", k_sb)
            # V transpose: 4x [128,128] bf16 PE transposes into one PSUM tile
            tps = psMM.tile([128, 4, 128], BF16, tag="mm",
                            name=f"tps_{hp}_{tt}")
            for st in range(4):
                nc.tensor.transpose(tps[:, st, :], vf[:, st * 128:(st + 1) * 128],
                                    id_sb[:])
            with tc.high_priority():
                for hl in range(2):
                    nc.vector.tensor_copy(
                        v_nat[hl][:, 4 * tt:4 * tt + 4, 0:Dh],
                        tps[:, :, hl * 64:hl * 64 + 64])

        def stage_b(hp, tt, q_sb, k_sb, v_nat, att_sb):
            """Causal attention for queries in tile tt, pipelined chunks.

            The QK moving operand reads q in (j,k')-major order (tau = j*32+k',
            t = 512*tt + 16*k' + j), free for the PE, so every downstream
            elementwise op is contiguous."""
            ts = slice(tt * TT, (tt + 1) * TT)
            njs = 4 * tt + 4
            acc = [psAC.tile([Dh + 1, TT], F32, tag="acc",
                             name=f"acc_{hp}_{tt}_{hl}") for hl in range(2)]
            pend = []  # (j, pb) entries awaiting AV, lag 2 behind QK
            for j in range(njs):
                sj = slice(j * SC, (j + 1) * SC)
                qk = psQK.tile([128, 2, TT], F32, tag="qk",
                               name=f"qk_{hp}_{tt}_{j}")
                for hl in range(2):
                    hb = hl * 64
                    nc.tensor.matmul(qk[:, hl], k_sb[hb:hb + 64, sj],
                                     q_sb[hb:hb + 64, ts],
                                     start=True, stop=True)
                pb = pbp.tile([128, 2, TT], BF16, tag="pb",
                              name=f"pb_{hp}_{tt}_{j}")
                c = j - 4 * tt
                if c < 0:
                    nc.scalar.activation(pb[:], qk[:], EXP, scale=SCALE)
                else:
                    off = 128 * c
                    if c == 0:
                        nc.scalar.activation(pb[:], qk[:], EXP, scale=SCALE)
                    else:
                        nc.gpsimd.memset(pb[:, :, 0:off], 0.0)
                        nc.scalar.activation(pb[:, :, off:], qk[:, :, off:],
                                             EXP, scale=SCALE)
                    nc.vector.tensor_mul(pb[:, :, off:off + 128],
                                         pb[:, :, off:off + 128],
                                         mask_sb[:])
                if len(pend) >= 2:
                    pj, ppb = pend.pop(0)
                    for hl in range(2):
                        nc.tensor.matmul(acc[hl][:], v_nat[hl][:, pj, :],
                                         ppb[:, hl],
                                         start=(pj == 0), stop=False)
                pend.append((j, pb))
            for pj, ppb in pend:
                for hl in range(2):
                    nc.tensor.matmul(acc[hl][:], v_nat[hl][:, pj, :],
                                     ppb[:, hl],
                                     start=(pj == 0), stop=(pj == njs - 1))
            # normalize into att_sb (layout [d, tt, tau])
            for hl in range(2):
                zrow = tmp.tile([1, TT], F32, tag="zrow",
                                name=f"zrow_{hp}_{tt}_{hl}")
                zi = tmp.tile([1, TT], F32, tag="zi", name=f"zi_{hp}_{tt}_{hl}")
                zb = tmp.tile([64, TT], F32, tag="zb", name=f"zb_{hp}_{tt}_{hl}")
                with tc.high_priority(offset=300):
                    nc.vector.tensor_copy(zrow[:], acc[hl][Dh:Dh + 1, :])
                    nc.vector.reciprocal_approx_fast(out=zi[:], in_=zrow[:])
                    nc.gpsimd.partition_broadcast(zb[:], zi[:], channels=64)
                    nc.vector.tensor_mul(att_sb[hl * 64:hl * 64 + 64, tt, :],
                                         acc[hl][0:Dh, :], zb[:])

        def repack(hp, att_sb, ahts):
            """att [d, tt, (k' j)] (t-linear) -> aht [two*64+d, cc, (tt k')]."""
            for hl in range(2):
                aht = ahpool.tile([128, 8, 128], BF16, tag="aht",
                                  name=f"aht_{hp}_{hl}")
                attv = att_sb[hl * 64:hl * 64 + 64].rearrange(
                    "d tt (k j) -> d j tt k", j=16)
                for two in range(2):
                    for cc in range(8):
                        nc.any.tensor_copy(
                            aht[two * 64:two * 64 + 64, cc, :].rearrange(
                                "d (tt k) -> d tt k", tt=4),
                            attv[:, 2 * cc + two])
                ahts.append(aht)

        def yproj(hp, hl, aht):
            blk = hp * 2 + hl
            for ot in range(2):
                ypss = psMM.tile([128, 512], F32, tag="mm",
                                 name=f"yps_{hp}_{hl}_{ot}")
                for cc in range(8):
                    nc.tensor.matmul(ypss[:], aht[:, cc, :],
                                     wt_sb[:, cc, ot * 512:(ot + 1) * 512],
                                     start=(cc == 0), stop=(cc == 7))
                yo = tmp.tile([128, 512], F32, tag="yo", bufs=4,
                              name=f"yo_{hp}_{hl}_{ot}")
                with tc.high_priority():
                    nc.any.tensor_copy(yo[:], ypss[:])
                nc.sync.dma_start(yblk[blk, ot], yo[:])

        prev = None  # (att_sb, ahts, hp) of previous head pair
        for hp in range(2):
            q_sb = qkpool.tile([128, T], BF16, tag="q", name=f"q_{hp}")
            k_sb = qkpool.tile([128, T], BF16, tag="k", name=f"k_{hp}")
            v_nat = [vpool.tile([128, T // SC, Dh + 1], BF16, tag="vnat",
                                name=f"vnat_{hp}_{hl}") for hl in range(2)]
            for hl in range(2):
                nc.gpsimd.memset(v_nat[hl][:, :, Dh:Dh + 1], 1.0)
            att_sb = attp.tile([128, 4, TT], BF16, tag="att",
                               name=f"att_{hp}")

            for tt in range(NTT):
                stage_a(hp, tt, q_sb, k_sb, v_nat)
            if prev is not None:
                # output projection of previous head pair; the scheduler
                # slots these into this head pair's ACT-bound windows
                patt, pahts, php = prev
                yproj(php, 0, pahts[0])
                yproj(php, 1, pahts[1])
                prev = None
            ahts = []
            for tt in range(NTT):
                stage_b(hp, tt, q_sb, k_sb, v_nat, att_sb)
            repack(hp, att_sb, ahts)
            prev = (att_sb, ahts, hp)
            if dbg and hp == 0:
                nc.sync.dma_start(qdbg[:], q_sb[:])
                nc.sync.dma_start(kdbg[:], k_sb[:])
                nc.sync.dma_start(vdbg[:], v_nat[0][:])
                nc.sync.dma_start(adbg[:], att_sb[:])
                nc.sync.dma_start(hdbg[:], ahts[0][:])

        patt, pahts, php = prev
        yproj(php, 0, pahts[0])
        yproj(php, 1, pahts[1])

    nc.compile()
    return nc


def _get_nc():
    global _compiled_nc
    if _compiled_nc is None:
        _compiled_nc = _build_nc()
    return _compiled_nc


def _host_tables():
    pos = np.arange(T, dtype=np.float32)[:, None]
    inv = np.exp(np.arange(0, Dh, 2, dtype=np.float32)
                 * (-math.log(10000.0) / Dh))
    ang = pos * inv                       # (T, 32)
    sin, cos = np.sin(ang), np.cos(ang)   # (T, 32)
    idx = np.arange(128) % HALF           # d % 32
    cos_ext = np.ascontiguousarray(cos[:, idx].T).astype(NPBF16)  # (128, T)
    sin_ext = np.ascontiguousarray(sin[:, idx].T).astype(NPBF16)

    R = np.zeros((128, 128), dtype=np.float32)
    for blk in (0, 64):
        for m in range(HALF):
            R[blk + m, blk + m + HALF] = -1.0
            R[blk + m + HALF, blk + m] = 1.0
    rt = np.ascontiguousarray(R.T).astype(NPBF16)

    s_i = np.arange(128)[:, None]
    t_i = np.arange(128)[None, :]
    mask01 = (t_i >= s_i).astype(np.float32).astype(NPBF16)
    mask2 = np.ascontiguousarray(
        np.broadcast_to(mask01[:, None, :], (128, 2, 128)))
    ident = np.eye(128, dtype=np.float32).astype(NPBF16)
    return cos_ext, sin_ext, rt, mask2, ident


def kernel(x, w_qkv, w_proj):
    x = np.asarray(x)
    w_qkv = np.asarray(w_qkv)
    w_proj = np.asarray(w_proj)
    nc = _get_nc()
    in_maps = build_in_maps(x, w_qkv, w_proj)
    res = run_bass_kernel_spmd(nc, in_maps, core_ids=list(range(NCORES)))
    y = np.zeros((B, T, C), dtype=np.float32)
    for c in range(NCORES):
        b, g = c // 4, c % 4
        yb = res.results[c]["yblk"]  # [4, 2, 128, 512]
        y[b, 512 * g:512 * g + 512, :] = yb.transpose(0, 2, 1, 3).reshape(512, C)
    return y


def build_in_maps(x, w_qkv, w_proj):
    cos_ext, sin_ext, rt, mask2, ident = _host_tables()
    wq4 = w_qkv.reshape(3, H, Dh, C)
    # w_proj^T packed [p, cc, o]
    wtr = np.ascontiguousarray(
        w_proj.T.reshape(8, 128, C).transpose(1, 0, 2)).astype(NPBF16)
    in_maps = []
    xr_cache = {}
    for c in range(NCORES):
        b, g = c // 4, c % 4
        hs = slice(4 * g, 4 * g + 4)
        wq = wq4[0, hs].reshape(CPC, C)
        wk = wq4[1, hs].reshape(CPC, C)
        wv = wq4[2, hs].reshape(CPC, C)
        # [p, g, cc, f]: wqkvr[p, g, cc, f] = w_g[f, cc*128+p] (g in q,k,v)
        wqkvr = np.ascontiguousarray(
            np.stack([wq, wk, wv], 0).reshape(3, CPC, 8, 128)
            .transpose(0, 3, 2, 1)).astype(NPBF16)
        if b not in xr_cache:
            xT = x[b].T  # (C, T)
            xr_cache[b] = np.ascontiguousarray(
                xT.reshape(8, 128, NTT, TT).transpose(2, 1, 0, 3)).astype(NPBF16)
        in_maps.append({
            "xr": xr_cache[b],
            "wqkvr": wqkvr,
            "wtr": wtr,
            "cosx": cos_ext, "sinx": sin_ext,
            "rt": rt, "idb": ident, "mask2": mask2,
        })
    return in_maps


# revision 53
# speedup vs baseline: 1.0363x; 1.0135x over previous
"""Multi-head attention (B=2,T=2048,C=1024,H=16,RoPE,causal) on 8 TRN2 cores.

Sharding: core c -> (batch b = c//4, head-group g = c%4, heads [4g,4g+4)).
Each core computes QKV projection for its 4 heads against x[b], RoPE,
causal attention in transposed-score layout [s, t], and the output
projection rows t' in [512g, 512g+512) of y[b] (the reference's
(B,H,T,Dh)->(B,T,C) reshape makes output blocks head-disjoint).

Schedule: stage A (proj+RoPE+vT) and stage B (attention) interleaved at
t-tile granularity (A0 A1 B0 A2 B1 A3 B2 B3) with a software-pipelined
QK->exp->AV chunk loop (QK[j+1] issued before AV[j]) so the PE never
waits on the activation engine.  All PSUM->SBUF staging copies run on
ACT/DVE/Pool chosen to balance engine load; exp instructions cover both
heads of a pair via strided APs.
"""
import math
import sys

sys.path.insert(0, '/opt/trn_rl_repo')
sys.path.insert(0, '/opt/pypackages')

import ml_dtypes
import numpy as np
from contextlib import ExitStack

import concourse.bass as bass  # noqa: F401
import concourse.tile as tile
from concourse import bacc, mybir
from concourse.bass_utils import run_bass_kernel_spmd

BF16 = mybir.dt.bfloat16
F32 = mybir.dt.float32
NPBF16 = ml_dtypes.bfloat16
EXP = mybir.ActivationFunctionType.Exp

B, T, C, H, Dh = 2, 2048, 1024, 16, 64
HALF = Dh // 2          # 32
NCORES = 8
HPC = 4                 # heads per core
CPC = HPC * Dh          # channels per core = 256
SCALE = 1.0 / math.sqrt(Dh)
TT = 512                # t-tile width
NTT = T // TT           # 4
SC = 128                # s-chunk width

_compiled_nc = None


def _calibrate_cost_model():
    """Calibrate the tile scheduler's cost model to measured HW speeds so
    its static schedule interleaves enough work to cover real ACT/DVE
    latencies (the stock model is optimistic and the fixed instruction
    order then stalls on hardware)."""
    from concourse import hw_specs
    spec = hw_specs.TRN2Spec
    if getattr(spec, "_mha_calibrated", False):
        return
    spec._mha_calibrated = True
    spec.CYCLE_T = {**spec.CYCLE_T,
                    mybir.EngineType.Activation: 1e9 / 0.90e9,
                    mybir.EngineType.DVE: 1e9 / 0.85e9}
    spec.SEM_DELAY = 120
    spec.PE_CYCLE = 1e9 / 2.1e9
    spec.DMA_CYCLE = spec.DMA_CYCLE * 1.5


def _build_nc(dbg=False):
    _calibrate_cost_model()
    nc = bacc.Bacc("TRN2", target_bir_lowering=False, debug=False)

    xr = nc.dram_tensor("xr", [NTT, 128, 8, TT], BF16, kind="ExternalInput").ap()
    wqkvr = nc.dram_tensor("wqkvr", [3, 128, 8, CPC], BF16,
                           kind="ExternalInput").ap()
    wtr = nc.dram_tensor("wtr", [128, 8, C], BF16, kind="ExternalInput").ap()
    cosx = nc.dram_tensor("cosx", [128, T], BF16, kind="ExternalInput").ap()
    sinx = nc.dram_tensor("sinx", [128, T], BF16, kind="ExternalInput").ap()
    rt = nc.dram_tensor("rt", [128, 128], BF16, kind="ExternalInput").ap()
    idb = nc.dram_tensor("idb", [128, 128], BF16, kind="ExternalInput").ap()
    mask2 = nc.dram_tensor("mask2", [128, 2, 128], BF16,
                           kind="ExternalInput").ap()
    yblk = nc.dram_tensor("yblk", [4, 2, 128, 512], F32, kind="ExternalOutput").ap()
    if dbg:
        qdbg = nc.dram_tensor("qdbg", [128, T], BF16, kind="ExternalOutput").ap()
        kdbg = nc.dram_tensor("kdbg", [128, T], BF16, kind="ExternalOutput").ap()
        vdbg = nc.dram_tensor("vdbg", [128, 16, Dh + 1], BF16,
                              kind="ExternalOutput").ap()
        adbg = nc.dram_tensor("adbg", [128, 4, TT], BF16,
                              kind="ExternalOutput").ap()
        hdbg = nc.dram_tensor("hdbg", [128, 8, 128], BF16,
                              kind="ExternalOutput").ap()

    with tile.TileContext(nc) as tc, ExitStack() as ctx:
        const = ctx.enter_context(tc.tile_pool(name="const", bufs=1))
        qkpool = ctx.enter_context(tc.tile_pool(name="qk", bufs=2))
        vpool = ctx.enter_context(tc.tile_pool(name="vnat", bufs=4))
        attp = ctx.enter_context(tc.tile_pool(name="attp", bufs=2))
        tmp = ctx.enter_context(tc.tile_pool(name="tmp", bufs=2))
        pbp = ctx.enter_context(tc.tile_pool(name="pbp", bufs=4))
        ahpool = ctx.enter_context(tc.tile_pool(name="ahp", bufs=2))
        psMM = ctx.enter_context(tc.tile_pool(name="psMM", bufs=2, space="PSUM"))
        psQK = ctx.enter_context(tc.tile_pool(name="psQK", bufs=2, space="PSUM"))
        psAC = ctx.enter_context(tc.tile_pool(name="psAC", bufs=2, space="PSUM"))

        # ---- input loads, in order of first use ----
        wqkv_sb = const.tile([128, 3, 8, CPC], BF16)
        nc.sync.dma_start(wqkv_sb[:, 0], wqkvr[0])
        x_sb = []
        for tt in range(NTT):
            x_sb.append(const.tile([128, 8, TT], BF16, name=f"x_sb{tt}"))
        nc.sync.dma_start(x_sb[0][:], xr[0])
        nc.sync.dma_start(wqkv_sb[:, 1], wqkvr[1])
        nc.sync.dma_start(wqkv_sb[:, 2], wqkvr[2])
        rt_sb = const.tile([128, 128], BF16)
        nc.sync.dma_start(rt_sb[:], rt[:])
        id_sb = const.tile([128, 128], BF16)
        nc.sync.dma_start(id_sb[:], idb[:])
        cos_sb = const.tile([128, T], BF16)
        nc.sync.dma_start(cos_sb[:], cosx[:])
        sin_sb = const.tile([128, T], BF16)
        nc.sync.dma_start(sin_sb[:], sinx[:])
        nc.sync.dma_start(x_sb[1][:], xr[1])
        mask_sb = const.tile([128, 2, 128], BF16)
        nc.sync.dma_start(mask_sb[:], mask2[:])
        nc.sync.dma_start(x_sb[2][:], xr[2])
        nc.sync.dma_start(x_sb[3][:], xr[3])
        wt_sb = const.tile([128, 8, C], BF16)
        nc.sync.dma_start(wt_sb[:], wtr[:])

        def stage_a(hp, tt, q_sb, k_sb, v_nat):
            """QKV projection + RoPE + V transpose for one t-tile."""
            ts = slice(tt * TT, (tt + 1) * TT)
            gps = {}
            gb = {}
            rot = {}
            # q/k projections, with RoPE staged behind each
            for grp, gi in (("q", 0), ("k", 1)):
                f0 = hp * 128
                gps[grp] = psMM.tile([128, TT], F32, tag="mm",
                                     name=f"gps_{grp}_{hp}_{tt}")
                for cc in range(8):
                    nc.tensor.matmul(gps[grp][:],
                                     wqkv_sb[:, gi, cc, f0:f0 + 128],
                                     x_sb[tt][:, cc, :],
                                     start=(cc == 0), stop=(cc == 7))
                # PSUM -> SBUF bf16 staging copy on ACT
                gb[grp] = tmp.tile([128, TT], BF16, tag=f"gb{grp}", name=f"gb_{grp}_{hp}_{tt}")
                with tc.high_priority():
                    nc.any.tensor_copy(gb[grp][:], gps[grp][:])
                if grp == "k":
                    # rot-q emitted here so PE has work while gb-k copies
                    rot["q"] = psMM.tile([128, TT], F32, tag="mm",
                                         name=f"rot_q_{hp}_{tt}")
                    nc.tensor.matmul(rot["q"][:], rt_sb[:], gb["q"][:],
                                     start=True, stop=True)
            # RoPE combine for q: dest = gb*cos (DVE 2x) + rot*sin (Pool)
            def rope_combine(grp, dest):
                m1 = tmp.tile([128, TT], BF16, tag="m1", name=f"m1_{grp}_{hp}_{tt}")
                m2 = tmp.tile([128, TT], BF16, tag="m2", name=f"m2_{grp}_{hp}_{tt}")
                with tc.high_priority():
                    nc.gpsimd.tensor_mul(m1[:], gb[grp][:], cos_sb[:, ts])
                    nc.vector.tensor_mul(m2[:], rot[grp][:], sin_sb[:, ts])
                    nc.vector.tensor_add(dest[:, ts], m1[:], m2[:])

            rope_combine("q", q_sb)
            # v projection (PE work covering the q-combine + rot-q release)
            f0 = hp * 128
            gps["v"] = psMM.tile([128, TT], F32, tag="mm",
                                 name=f"gps_v_{hp}_{tt}")
            for cc in range(8):
                nc.tensor.matmul(gps["v"][:], wqkv_sb[:, 2, cc, f0:f0 + 128],
                                 x_sb[tt][:, cc, :],
                                 start=(cc == 0), stop=(cc == 7))
            vf = tmp.tile([128, TT], BF16, tag="vf", name=f"vf_{hp}_{tt}")
            with tc.high_priority():
                nc.vector.tensor_copy(vf[:], gps["v"][:])
            rot["k"] = psMM.tile([128, TT], F32, tag="mm",
                                 name=f"rot_k_{hp}_{tt}")
            nc.tensor.matmul(rot["k"][:], rt_sb[:], gb["k"][:],
                             start=True, stop=True)
            rope_combine("k", k_sb)
            # V transpose: 4x [128,128] bf16 PE transposes into one PSUM tile
            tps = psMM.tile([128, 4, 128], BF16, tag="mm",
                            name=f"tps_{hp}_{tt}")
            for st in range(4):
                nc.tensor.transpose(tps[:, st, :], vf[:, st * 128:(st + 1) * 128],
                                    id_sb[:])
            with tc.high_priority():
                for hl in range(2):
                    nc.vector.tensor_copy(
                        v_nat[hl][:, 4 * tt:4 * tt + 4, 0:Dh],
                        tps[:, :, hl * 64:hl * 64 + 64])

        def stage_b(hp, tt, q_sb, k_sb, v_nat, att_sb):
            """Causal attention for queries in tile tt, pipelined chunks.

            The QK moving operand reads q in (j,k')-major order (tau = j*32+k',
            t = 512*tt + 16*k' + j), free for the PE, so every downstream
            elementwise op is contiguous."""
            ts = slice(tt * TT, (tt + 1) * TT)
            njs = 4 * tt + 4
            acc = [psAC.tile([Dh + 1, TT], F32, tag="acc",
                             name=f"acc_{hp}_{tt}_{hl}") for hl in range(2)]
            pend = []  # (j, pb) entries awaiting AV, lag 2 behind QK
            for j in range(njs):
                sj = slice(j * SC, (j + 1) * SC)
                qk = psQK.tile([128, 2, TT], F32, tag="qk",
                               name=f"qk_{hp}_{tt}_{j}")
                for hl in range(2):
                    hb = hl * 64
                    nc.tensor.matmul(qk[:, hl], k_sb[hb:hb + 64, sj],
                                     q_sb[hb:hb + 64, ts],
                                     start=True, stop=True)
                pb = pbp.tile([128, 2, TT], BF16, tag="pb",
                              name=f"pb_{hp}_{tt}_{j}")
                c = j - 4 * tt
                if c < 0:
                    nc.scalar.activation(pb[:], qk[:], EXP, scale=SCALE)
                else:
                    off = 128 * c
                    if c == 0:
                        nc.scalar.activation(pb[:], qk[:], EXP, scale=SCALE)
                    else:
                        nc.gpsimd.memset(pb[:, :, 0:off], 0.0)
                        nc.scalar.activation(pb[:, :, off:], qk[:, :, off:],
                                             EXP, scale=SCALE)
                    nc.vector.tensor_mul(pb[:, :, off:off + 128],
                                         pb[:, :, off:off + 128],
                                         mask_sb[:])
                if len(pend) >= 2:
                    pj, ppb = pend.pop(0)
                    for hl in range(2):
                        nc.tensor.matmul(acc[hl][:], v_nat[hl][:, pj, :],
                                         ppb[:, hl],
                                         start=(pj == 0), stop=False)
                pend.append((j, pb))
            for pj, ppb in pend:
                for hl in range(2):
                    nc.tensor.matmul(acc[hl][:], v_nat[hl][:, pj, :],
                                     ppb[:, hl],
                                     start=(pj == 0), stop=(pj == njs - 1))
            # normalize into att_sb (layout [d, tt, tau])
            for hl in range(2):
                zrow = tmp.tile([1, TT], F32, tag="zrow",
                                name=f"zrow_{hp}_{tt}_{hl}")
                zi = tmp.tile([1, TT], F32, tag="zi", name=f"zi_{hp}_{tt}_{hl}")
                zb = tmp.tile([64, TT], F32, tag="zb", name=f"zb_{hp}_{tt}_{hl}")
                with tc.high_priority(offset=300):
                    nc.vector.tensor_copy(zrow[:], acc[hl][Dh:Dh + 1, :])
                    nc.vector.reciprocal_approx_fast(out=zi[:], in_=zrow[:])
                    nc.gpsimd.partition_broadcast(zb[:], zi[:], channels=64)
                    nc.vector.tensor_mul(att_sb[hl * 64:hl * 64 + 64, tt, :],
                                         acc[hl][0:Dh, :], zb[:])

        def repack(hp, att_sb, ahts):
            """att [d, tt, (k' j)] (t-linear) -> aht [two*64+d, cc, (tt k')]."""
            for hl in range(2):
                aht = ahpool.tile([128, 8, 128], BF16, tag="aht",
                                  name=f"aht_{hp}_{hl}")
                attv = att_sb[hl * 64:hl * 64 + 64].rearrange(
                    "d tt (k j) -> d j tt k", j=16)
                for two in range(2):
                    for cc in range(8):
                        nc.any.tensor_copy(
                            aht[two * 64:two * 64 + 64, cc, :].rearrange(
                                "d (tt k) -> d tt k", tt=4),
                            attv[:, 2 * cc + two])
                ahts.append(aht)

        def yproj(hp, hl, aht):
            blk = hp * 2 + hl
            for ot in range(2):
                ypss = psMM.tile([128, 512], F32, tag="mm",
                                 name=f"yps_{hp}_{hl}_{ot}")
                for cc in range(8):
                    nc.tensor.matmul(ypss[:], aht[:, cc, :],
                                     wt_sb[:, cc, ot * 512:(ot + 1) * 512],
                                     start=(cc == 0), stop=(cc == 7))
                yo = tmp.tile([128, 512], F32, tag="yo", bufs=4,
                              name=f"yo_{hp}_{hl}_{ot}")
                with tc.high_priority():
                    nc.any.tensor_copy(yo[:], ypss[:])
                nc.sync.dma_start(yblk[blk, ot], yo[:])

        prev = None  # (att_sb, ahts, hp) of previous head pair
        for hp in range(2):
            q_sb = qkpool.tile([128, T], BF16, tag="q", name=f"q_{hp}")
            k_sb = qkpool.tile([128, T], BF16, tag="k", name=f"k_{hp}")
            v_nat = [vpool.tile([128, T // SC, Dh + 1], BF16, tag="vnat",
                                name=f"vnat_{hp}_{hl}") for hl in range(2)]
            for hl in range(2):
                nc.gpsimd.memset(v_nat[hl][:, :, Dh:Dh + 1], 1.0)
            att_sb = attp.tile([128, 4, TT], BF16, tag="att",
                               name=f"att_{hp}")

            for tt in range(NTT):
                stage_a(hp, tt, q_sb, k_sb, v_nat)
            if prev is not None:
                # output projection of previous head pair; the scheduler
                # slots these into this head pair's ACT-bound windows
                patt, pahts, php = prev
                yproj(php, 0, pahts[0])
                yproj(php, 1, pahts[1])
                prev = None
            ahts = []
            for tt in range(NTT):
                stage_b(hp, tt, q_sb, k_sb, v_nat, att_sb)
            repack(hp, att_sb, ahts)
            prev = (att_sb, ahts, hp)
            if dbg and hp == 0:
                nc.sync.dma_start(qdbg[:], q_sb[:])
                nc.sync.dma_start(kdbg[:], k_sb[:])
                nc.sync.dma_start(vdbg[:], v_nat[0][:])
                nc.sync.dma_start(adbg[:], att_sb[:])
                nc.sync.dma_start(hdbg[:], ahts[0][:])

        patt, pahts, php = prev
        yproj(php, 0, pahts[0])
        yproj(php, 1, pahts[1])

    nc.compile()
    return nc


def _get_nc():
    global _compiled_nc
    if _compiled_nc is None:
        _compiled_nc = _build_nc()
    return _compiled_nc


def _host_tables():
    pos = np.arange(T, dtype=np.float32)[:, None]
    inv = np.exp(np.arange(0, Dh, 2, dtype=np.float32)
                 * (-math.log(10000.0) / Dh))
    ang = pos * inv                       # (T, 32)
    sin, cos = np.sin(ang), np.cos(ang)   # (T, 32)
    idx = np.arange(128) % HALF           # d % 32
    cos_ext = np.ascontiguousarray(cos[:, idx].T).astype(NPBF16)  # (128, T)
    sin_ext = np.ascontiguousarray(sin[:, idx].T).astype(NPBF16)

    R = np.zeros((128, 128), dtype=np.float32)
    for blk in (0, 64):
        for m in range(HALF):
            R[blk + m, blk + m + HALF] = -1.0
            R[blk + m + HALF, blk + m] = 1.0
    rt = np.ascontiguousarray(R.T).astype(NPBF16)

    s_i = np.arange(128)[:, None]
    t_i = np.arange(128)[None, :]
    mask01 = (t_i >= s_i).astype(np.float32).astype(NPBF16)
    mask2 = np.ascontiguousarray(
        np.broadcast_to(mask01[:, None, :], (128, 2, 128)))
    ident = np.eye(128, dtype=np.float32).astype(NPBF16)
    return cos_ext, sin_ext, rt, mask2, ident


def kernel(x, w_qkv, w_proj):
    x = np.asarray(x)
    w_qkv = np.asarray(w_qkv)
    w_proj = np.asarray(w_proj)
    nc = _get_nc()
    in_maps = build_in_maps(x, w_qkv, w_proj)
    res = run_bass_kernel_spmd(nc, in_maps, core_ids=list(range(NCORES)))
    y = np.zeros((B, T, C), dtype=np.float32)
    for c in range(NCORES):
        b, g = c // 4, c % 4
        yb = res.results[c]["yblk"]  # [4, 2, 128, 512]
        y[b, 512 * g:512 * g + 512, :] = yb.transpose(0, 2, 1, 3).reshape(512, C)
    return y


def build_in_maps(x, w_qkv, w_proj):
    cos_ext, sin_ext, rt, mask2, ident = _host_tables()
    wq4 = w_qkv.reshape(3, H, Dh, C)
    # w_proj^T packed [p, cc, o]
    wtr = np.ascontiguousarray(
        w_proj.T.reshape(8, 128, C).transpose(1, 0, 2)).astype(NPBF16)
    in_maps = []
    xr_cache = {}
    for c in range(NCORES):
        b, g = c // 4, c % 4
        hs = slice(4 * g, 4 * g + 4)
        wq = wq4[0, hs].reshape(CPC, C)
        wk = wq4[1, hs].reshape(CPC, C)
        wv = wq4[2, hs].reshape(CPC, C)
        # [p, g, cc, f]: wqkvr[p, g, cc, f] = w_g[f, cc*128+p] (g in q,k,v)
        wqkvr = np.ascontiguousarray(
            np.stack([wq, wk, wv], 0).reshape(3, CPC, 8, 128)
            .transpose(0, 3, 2, 1)).astype(NPBF16)
        if b not in xr_cache:
            xT = x[b].T  # (C, T)
            xr_cache[b] = np.ascontiguousarray(
                xT.reshape(8, 128, NTT, TT).transpose(2, 1, 0, 3)).astype(NPBF16)
        in_maps.append({
            "xr": xr_cache[b],
            "wqkvr": wqkvr,
            "wtr": wtr,
            "cosx": cos_ext, "sinx": sin_ext,
            "rt": rt, "idb": ident, "mask2": mask2,
        })
    return in_maps


# revision 54
# speedup vs baseline: 1.0520x; 1.0151x over previous
"""Multi-head attention (B=2,T=2048,C=1024,H=16,RoPE,causal) on 8 TRN2 cores.

Sharding: core c -> (batch b = c//4, head-group g = c%4, heads [4g,4g+4)).
Each core computes QKV projection for its 4 heads against x[b], RoPE,
causal attention in transposed-score layout [s, t], and the output
projection rows t' in [512g, 512g+512) of y[b] (the reference's
(B,H,T,Dh)->(B,T,C) reshape makes output blocks head-disjoint).

Schedule: stage A (proj+RoPE+vT) and stage B (attention) interleaved at
t-tile granularity (A0 A1 B0 A2 B1 A3 B2 B3) with a software-pipelined
QK->exp->AV chunk loop (QK[j+1] issued before AV[j]) so the PE never
waits on the activation engine.  All PSUM->SBUF staging copies run on
ACT/DVE/Pool chosen to balance engine load; exp instructions cover both
heads of a pair via strided APs.
"""
import math
import sys

sys.path.insert(0, '/opt/trn_rl_repo')
sys.path.insert(0, '/opt/pypackages')

import ml_dtypes
import numpy as np
from contextlib import ExitStack

import concourse.bass as bass  # noqa: F401
import concourse.tile as tile
from concourse import bacc, mybir
from concourse.bass_utils import run_bass_kernel_spmd

BF16 = mybir.dt.bfloat16
F32 = mybir.dt.float32
NPBF16 = ml_dtypes.bfloat16
EXP = mybir.ActivationFunctionType.Exp

B, T, C, H, Dh = 2, 2048, 1024, 16, 64
HALF = Dh // 2          # 32
NCORES = 8
HPC = 4                 # heads per core
CPC = HPC * Dh          # channels per core = 256
SCALE = 1.0 / math.sqrt(Dh)
TT = 512                # t-tile width
NTT = T // TT           # 4
SC = 128                # s-chunk width

_compiled_nc = None


def _calibrate_cost_model():
    """Calibrate the tile scheduler's cost model to measured HW speeds so
    its static schedule interleaves enough work to cover real ACT/DVE
    latencies (the stock model is optimistic and the fixed instruction
    order then stalls on hardware)."""
    from concourse import hw_specs
    spec = hw_specs.TRN2Spec
    if getattr(spec, "_mha_calibrated", False):
        return
    spec._mha_calibrated = True
    spec.CYCLE_T = {**spec.CYCLE_T,
                    mybir.EngineType.Activation: 1e9 / 0.90e9,
                    mybir.EngineType.DVE: 1e9 / 0.85e9}
    spec.SEM_DELAY = 130
    spec.PE_CYCLE = 1e9 / 2.1e9
    spec.DMA_CYCLE = spec.DMA_CYCLE * 1.5


def _build_nc(dbg=False):
    _calibrate_cost_model()
    nc = bacc.Bacc("TRN2", target_bir_lowering=False, debug=False)

    xr = nc.dram_tensor("xr", [NTT, 128, 8, TT], BF16, kind="ExternalInput").ap()
    wqkvr = nc.dram_tensor("wqkvr", [3, 128, 8, CPC], BF16,
                           kind="ExternalInput").ap()
    wtr = nc.dram_tensor("wtr", [128, 8, C], BF16, kind="ExternalInput").ap()
    cosx = nc.dram_tensor("cosx", [128, T], BF16, kind="ExternalInput").ap()
    sinx = nc.dram_tensor("sinx", [128, T], BF16, kind="ExternalInput").ap()
    rt = nc.dram_tensor("rt", [128, 128], BF16, kind="ExternalInput").ap()
    idb = nc.dram_tensor("idb", [128, 128], BF16, kind="ExternalInput").ap()
    mask2 = nc.dram_tensor("mask2", [128, 2, 128], BF16,
                           kind="ExternalInput").ap()
    yblk = nc.dram_tensor("yblk", [4, 2, 128, 512], F32, kind="ExternalOutput").ap()
    if dbg:
        qdbg = nc.dram_tensor("qdbg", [128, T], BF16, kind="ExternalOutput").ap()
        kdbg = nc.dram_tensor("kdbg", [128, T], BF16, kind="ExternalOutput").ap()
        vdbg = nc.dram_tensor("vdbg", [128, 16, Dh + 1], BF16,
                              kind="ExternalOutput").ap()
        adbg = nc.dram_tensor("adbg", [128, 4, TT], BF16,
                              kind="ExternalOutput").ap()
        hdbg = nc.dram_tensor("hdbg", [128, 8, 128], BF16,
                              kind="ExternalOutput").ap()

    with tile.TileContext(nc) as tc, ExitStack() as ctx:
        const = ctx.enter_context(tc.tile_pool(name="const", bufs=1))
        qkpool = ctx.enter_context(tc.tile_pool(name="qk", bufs=2))
        vpool = ctx.enter_context(tc.tile_pool(name="vnat", bufs=4))
        attp = ctx.enter_context(tc.tile_pool(name="attp", bufs=2))
        tmp = ctx.enter_context(tc.tile_pool(name="tmp", bufs=2))
        pbp = ctx.enter_context(tc.tile_pool(name="pbp", bufs=4))
        ahpool = ctx.enter_context(tc.tile_pool(name="ahp", bufs=2))
        psMM = ctx.enter_context(tc.tile_pool(name="psMM", bufs=2, space="PSUM"))
        psQK = ctx.enter_context(tc.tile_pool(name="psQK", bufs=2, space="PSUM"))
        psAC = ctx.enter_context(tc.tile_pool(name="psAC", bufs=2, space="PSUM"))

        # ---- input loads, in order of first use ----
        wqkv_sb = const.tile([128, 3, 8, CPC], BF16)
        nc.sync.dma_start(wqkv_sb[:, 0], wqkvr[0])
        x_sb = []
        for tt in range(NTT):
            x_sb.append(const.tile([128, 8, TT], BF16, name=f"x_sb{tt}"))
        nc.sync.dma_start(x_sb[0][:], xr[0])
        nc.sync.dma_start(wqkv_sb[:, 1], wqkvr[1])
        nc.sync.dma_start(wqkv_sb[:, 2], wqkvr[2])
        rt_sb = const.tile([128, 128], BF16)
        nc.sync.dma_start(rt_sb[:], rt[:])
        id_sb = const.tile([128, 128], BF16)
        nc.sync.dma_start(id_sb[:], idb[:])
        cos_sb = const.tile([128, T], BF16)
        nc.sync.dma_start(cos_sb[:], cosx[:])
        sin_sb = const.tile([128, T], BF16)
        nc.sync.dma_start(sin_sb[:], sinx[:])
        nc.sync.dma_start(x_sb[1][:], xr[1])
        mask_sb = const.tile([128, 2, 128], BF16)
        nc.sync.dma_start(mask_sb[:], mask2[:])
        nc.sync.dma_start(x_sb[2][:], xr[2])
        nc.sync.dma_start(x_sb[3][:], xr[3])
        wt_sb = const.tile([128, 8, C], BF16)
        nc.sync.dma_start(wt_sb[:], wtr[:])

        def stage_a(hp, tt, q_sb, k_sb, v_nat):
            """QKV projection + RoPE + V transpose for one t-tile."""
            ts = slice(tt * TT, (tt + 1) * TT)
            gps = {}
            gb = {}
            rot = {}
            # q/k projections, with RoPE staged behind each
            for grp, gi in (("q", 0), ("k", 1)):
                f0 = hp * 128
                gps[grp] = psMM.tile([128, TT], F32, tag="mm",
                                     name=f"gps_{grp}_{hp}_{tt}")
                for cc in range(8):
                    nc.tensor.matmul(gps[grp][:],
                                     wqkv_sb[:, gi, cc, f0:f0 + 128],
                                     x_sb[tt][:, cc, :],
                                     start=(cc == 0), stop=(cc == 7))
                # PSUM -> SBUF bf16 staging copy on ACT
                gb[grp] = tmp.tile([128, TT], BF16, tag=f"gb{grp}", name=f"gb_{grp}_{hp}_{tt}")
                with tc.high_priority():
                    nc.any.tensor_copy(gb[grp][:], gps[grp][:])
                if grp == "k":
                    # rot-q emitted here so PE has work while gb-k copies
                    rot["q"] = psMM.tile([128, TT], F32, tag="mm",
                                         name=f"rot_q_{hp}_{tt}")
                    nc.tensor.matmul(rot["q"][:], rt_sb[:], gb["q"][:],
                                     start=True, stop=True)
            # RoPE combine for q: dest = gb*cos (DVE 2x) + rot*sin (Pool)
            def rope_combine(grp, dest):
                m1 = tmp.tile([128, TT], BF16, tag="m1", name=f"m1_{grp}_{hp}_{tt}")
                m2 = tmp.tile([128, TT], BF16, tag="m2", name=f"m2_{grp}_{hp}_{tt}")
                with tc.high_priority():
                    nc.gpsimd.tensor_mul(m1[:], gb[grp][:], cos_sb[:, ts])
                    nc.vector.tensor_mul(m2[:], rot[grp][:], sin_sb[:, ts])
                    nc.vector.tensor_add(dest[:, ts], m1[:], m2[:])

            rope_combine("q", q_sb)
            # v projection (PE work covering the q-combine + rot-q release)
            f0 = hp * 128
            gps["v"] = psMM.tile([128, TT], F32, tag="mm",
                                 name=f"gps_v_{hp}_{tt}")
            for cc in range(8):
                nc.tensor.matmul(gps["v"][:], wqkv_sb[:, 2, cc, f0:f0 + 128],
                                 x_sb[tt][:, cc, :],
                                 start=(cc == 0), stop=(cc == 7))
            vf = tmp.tile([128, TT], BF16, tag="vf", name=f"vf_{hp}_{tt}")
            with tc.high_priority():
                nc.vector.tensor_copy(vf[:], gps["v"][:])
            rot["k"] = psMM.tile([128, TT], F32, tag="mm",
                                 name=f"rot_k_{hp}_{tt}")
            nc.tensor.matmul(rot["k"][:], rt_sb[:], gb["k"][:],
                             start=True, stop=True)
            rope_combine("k", k_sb)
            # V transpose: 4x [128,128] bf16 PE transposes into one PSUM tile
            tps = psMM.tile([128, 4, 128], BF16, tag="mm",
                            name=f"tps_{hp}_{tt}")
            for st in range(4):
                nc.tensor.transpose(tps[:, st, :], vf[:, st * 128:(st + 1) * 128],
                                    id_sb[:])
            with tc.high_priority():
                for hl in range(2):
                    nc.vector.tensor_copy(
                        v_nat[hl][:, 4 * tt:4 * tt + 4, 0:Dh],
                        tps[:, :, hl * 64:hl * 64 + 64])

        def stage_b(hp, tt, q_sb, k_sb, v_nat, att_sb):
            """Causal attention for queries in tile tt, pipelined chunks.

            The QK moving operand reads q in (j,k')-major order (tau = j*32+k',
            t = 512*tt + 16*k' + j), free for the PE, so every downstream
            elementwise op is contiguous."""
            ts = slice(tt * TT, (tt + 1) * TT)
            njs = 4 * tt + 4
            acc = [psAC.tile([Dh + 1, TT], F32, tag="acc",
                             name=f"acc_{hp}_{tt}_{hl}") for hl in range(2)]
            pend = []  # (j, pb) entries awaiting AV, lag 2 behind QK
            for j in range(njs):
                sj = slice(j * SC, (j + 1) * SC)
                qk = psQK.tile([128, 2, TT], F32, tag="qk",
                               name=f"qk_{hp}_{tt}_{j}")
                for hl in range(2):
                    hb = hl * 64
                    nc.tensor.matmul(qk[:, hl], k_sb[hb:hb + 64, sj],
                                     q_sb[hb:hb + 64, ts],
                                     start=True, stop=True)
                pb = pbp.tile([128, 2, TT], BF16, tag="pb",
                              name=f"pb_{hp}_{tt}_{j}")
                c = j - 4 * tt
                if c < 0:
                    nc.scalar.activation(pb[:], qk[:], EXP, scale=SCALE)
                else:
                    off = 128 * c
                    if c == 0:
                        nc.scalar.activation(pb[:], qk[:], EXP, scale=SCALE)
                    else:
                        nc.gpsimd.memset(pb[:, :, 0:off], 0.0)
                        nc.scalar.activation(pb[:, :, off:], qk[:, :, off:],
                                             EXP, scale=SCALE)
                    nc.vector.tensor_mul(pb[:, :, off:off + 128],
                                         pb[:, :, off:off + 128],
                                         mask_sb[:])
                if len(pend) >= 2:
                    pj, ppb = pend.pop(0)
                    for hl in range(2):
                        nc.tensor.matmul(acc[hl][:], v_nat[hl][:, pj, :],
                                         ppb[:, hl],
                                         start=(pj == 0), stop=False)
                pend.append((j, pb))
            for pj, ppb in pend:
                for hl in range(2):
                    nc.tensor.matmul(acc[hl][:], v_nat[hl][:, pj, :],
                                     ppb[:, hl],
                                     start=(pj == 0), stop=(pj == njs - 1))
            # normalize into att_sb (layout [d, tt, tau])
            for hl in range(2):
                zrow = tmp.tile([1, TT], F32, tag="zrow",
                                name=f"zrow_{hp}_{tt}_{hl}")
                zi = tmp.tile([1, TT], F32, tag="zi", name=f"zi_{hp}_{tt}_{hl}")
                zb = tmp.tile([64, TT], F32, tag="zb", name=f"zb_{hp}_{tt}_{hl}")
                with tc.high_priority(offset=300):
                    nc.vector.tensor_copy(zrow[:], acc[hl][Dh:Dh + 1, :])
                    nc.vector.reciprocal_approx_fast(out=zi[:], in_=zrow[:])
                    nc.gpsimd.partition_broadcast(zb[:], zi[:], channels=64)
                    nc.vector.tensor_mul(att_sb[hl * 64:hl * 64 + 64, tt, :],
                                         acc[hl][0:Dh, :], zb[:])

        def repack(hp, att_sb, ahts):
            """att [d, tt, (k' j)] (t-linear) -> aht [two*64+d, cc, (tt k')]."""
            for hl in range(2):
                aht = ahpool.tile([128, 8, 128], BF16, tag="aht",
                                  name=f"aht_{hp}_{hl}")
                attv = att_sb[hl * 64:hl * 64 + 64].rearrange(
                    "d tt (k j) -> d j tt k", j=16)
                for two in range(2):
                    for cc in range(8):
                        nc.any.tensor_copy(
                            aht[two * 64:two * 64 + 64, cc, :].rearrange(
                                "d (tt k) -> d tt k", tt=4),
                            attv[:, 2 * cc + two])
                ahts.append(aht)

        def yproj(hp, hl, aht):
            blk = hp * 2 + hl
            for ot in range(2):
                ypss = psMM.tile([128, 512], F32, tag="mm",
                                 name=f"yps_{hp}_{hl}_{ot}")
                for cc in range(8):
                    nc.tensor.matmul(ypss[:], aht[:, cc, :],
                                     wt_sb[:, cc, ot * 512:(ot + 1) * 512],
                                     start=(cc == 0), stop=(cc == 7))
                yo = tmp.tile([128, 512], F32, tag="yo", bufs=4,
                              name=f"yo_{hp}_{hl}_{ot}")
                with tc.high_priority():
                    nc.any.tensor_copy(yo[:], ypss[:])
                nc.sync.dma_start(yblk[blk, ot], yo[:])

        prev = None  # (att_sb, ahts, hp) of previous head pair
        for hp in range(2):
            q_sb = qkpool.tile([128, T], BF16, tag="q", name=f"q_{hp}")
            k_sb = qkpool.tile([128, T], BF16, tag="k", name=f"k_{hp}")
            v_nat = [vpool.tile([128, T // SC, Dh + 1], BF16, tag="vnat",
                                name=f"vnat_{hp}_{hl}") for hl in range(2)]
            for hl in range(2):
                nc.gpsimd.memset(v_nat[hl][:, :, Dh:Dh + 1], 1.0)
            att_sb = attp.tile([128, 4, TT], BF16, tag="att",
                               name=f"att_{hp}")

            for tt in range(NTT):
                stage_a(hp, tt, q_sb, k_sb, v_nat)
            if prev is not None:
                # output projection of previous head pair; the scheduler
                # slots these into this head pair's ACT-bound windows
                patt, pahts, php = prev
                yproj(php, 0, pahts[0])
                yproj(php, 1, pahts[1])
                prev = None
            ahts = []
            for tt in range(NTT):
                stage_b(hp, tt, q_sb, k_sb, v_nat, att_sb)
            repack(hp, att_sb, ahts)
            prev = (att_sb, ahts, hp)
            if dbg and hp == 0:
                nc.sync.dma_start(qdbg[:], q_sb[:])
                nc.sync.dma_start(kdbg[:], k_sb[:])
                nc.sync.dma_start(vdbg[:], v_nat[0][:])
                nc.sync.dma_start(adbg[:], att_sb[:])
                nc.sync.dma_start(hdbg[:], ahts[0][:])

        patt, pahts, php = prev
        yproj(php, 0, pahts[0])
        yproj(php, 1, pahts[1])

    nc.compile()
    return nc


def _get_nc():
    global _compiled_nc
    if _compiled_nc is None:
        _compiled_nc = _build_nc()
    return _compiled_nc


def _host_tables():
    pos = np.arange(T, dtype=np.float32)[:, None]
    inv = np.exp(np.arange(0, Dh, 2, dtype=np.float32)
                 * (-math.log(10000.0) / Dh))
    ang = pos * inv                       # (T, 32)
    sin, cos = np.sin(ang), np.cos(ang)   # (T, 32)
    idx = np.arange(128) % HALF           # d % 32
    cos_ext = np.ascontiguousarray(cos[:, idx].T).astype(NPBF16)  # (128, T)
    sin_ext = np.ascontiguousarray(sin[:, idx].T).astype(NPBF16)

    R = np.zeros((128, 128), dtype=np.float32)
    for blk in (0, 64):
        for m in range(HALF):
            R[blk + m, blk + m + HALF] = -1.0
            R[blk + m + HALF, blk + m] = 1.0
    rt = np.ascontiguousarray(R.T).astype(NPBF16)

    s_i = np.arange(128)[:, None]
    t_i = np.arange(128)[None, :]
    mask01 = (t_i >= s_i).astype(np.float32).astype(NPBF16)
    mask2 = np.ascontiguousarray(
        np.broadcast_to(mask01[:, None, :], (128, 2, 128)))
    ident = np.eye(128, dtype=np.float32).astype(NPBF16)
    return cos_ext, sin_ext, rt, mask2, ident


def kernel(x, w_qkv, w_proj):
    x = np.asarray(x)
    w_qkv = np.asarray(w_qkv)
    w_proj = np.asarray(w_proj)
    nc = _get_nc()
    in_maps = build_in_maps(x, w_qkv, w_proj)
    res = run_bass_kernel_spmd(nc, in_maps, core_ids=list(range(NCORES)))
    y = np.zeros((B, T, C), dtype=np.float32)
    for c in range(NCORES):
        b, g = c // 4, c % 4
        yb = res.results[c]["yblk"]  # [4, 2, 128, 512]
        y[b, 512 * g:512 * g + 512, :] = yb.transpose(0, 2, 1, 3).reshape(512, C)
    return y


def build_in_maps(x, w_qkv, w_proj):
    cos_ext, sin_ext, rt, mask2, ident = _host_tables()
    wq4 = w_qkv.reshape(3, H, Dh, C)
    # w_proj^T packed [p, cc, o]
    wtr = np.ascontiguousarray(
        w_proj.T.reshape(8, 128, C).transpose(1, 0, 2)).astype(NPBF16)
    in_maps = []
    xr_cache = {}
    for c in range(NCORES):
        b, g = c // 4, c % 4
        hs = slice(4 * g, 4 * g + 4)
        wq = wq4[0, hs].reshape(CPC, C)
        wk = wq4[1, hs].reshape(CPC, C)
        wv = wq4[2, hs].reshape(CPC, C)
        # [p, g, cc, f]: wqkvr[p, g, cc, f] = w_g[f, cc*128+p] (g in q,k,v)
        wqkvr = np.ascontiguousarray(
            np.stack([wq, wk, wv], 0).reshape(3, CPC, 8, 128)
            .transpose(0, 3, 2, 1)).astype(NPBF16)
        if b not in xr_cache:
            xT = x[b].T  # (C, T)
            xr_cache[b] = np.ascontiguousarray(
                xT.reshape(8, 128, NTT, TT).transpose(2, 1, 0, 3)).astype(NPBF16)
        in_maps.append({
            "xr": xr_cache[b],
            "wqkvr": wqkvr,
            "wtr": wtr,
            "cosx": cos_ext, "sinx": sin_ext,
            "rt": rt, "idb": ident, "mask2": mask2,
        })
    return in_maps


# revision 55
# speedup vs baseline: 1.0604x; 1.0080x over previous
"""Multi-head attention (B=2,T=2048,C=1024,H=16,RoPE,causal) on 8 TRN2 cores.

Sharding: core c -> (batch b = c//4, head-group g = c%4, heads [4g,4g+4)).
Each core computes QKV projection for its 4 heads against x[b], RoPE,
causal attention in transposed-score layout [s, t], and the output
projection rows t' in [512g, 512g+512) of y[b] (the reference's
(B,H,T,Dh)->(B,T,C) reshape makes output blocks head-disjoint).

Schedule: stage A (proj+RoPE+vT) and stage B (attention) interleaved at
t-tile granularity (A0 A1 B0 A2 B1 A3 B2 B3) with a software-pipelined
QK->exp->AV chunk loop (QK[j+1] issued before AV[j]) so the PE never
waits on the activation engine.  All PSUM->SBUF staging copies run on
ACT/DVE/Pool chosen to balance engine load; exp instructions cover both
heads of a pair via strided APs.
"""
import math
import sys

sys.path.insert(0, '/opt/trn_rl_repo')
sys.path.insert(0, '/opt/pypackages')

import ml_dtypes
import numpy as np
from contextlib import ExitStack

import concourse.bass as bass  # noqa: F401
import concourse.tile as tile
from concourse import bacc, mybir
from concourse.bass_utils import run_bass_kernel_spmd

BF16 = mybir.dt.bfloat16
F32 = mybir.dt.float32
NPBF16 = ml_dtypes.bfloat16
EXP = mybir.ActivationFunctionType.Exp

B, T, C, H, Dh = 2, 2048, 1024, 16, 64
HALF = Dh // 2          # 32
NCORES = 8
HPC = 4                 # heads per core
CPC = HPC * Dh          # channels per core = 256
SCALE = 1.0 / math.sqrt(Dh)
TT = 512                # t-tile width
NTT = T // TT           # 4
SC = 128                # s-chunk width

_compiled_nc = None


def _calibrate_cost_model():
    """Calibrate the tile scheduler's cost model to measured HW speeds so
    its static schedule interleaves enough work to cover real ACT/DVE
    latencies (the stock model is optimistic and the fixed instruction
    order then stalls on hardware)."""
    from concourse import hw_specs
    spec = hw_specs.TRN2Spec
    if getattr(spec, "_mha_calibrated", False):
        return
    spec._mha_calibrated = True
    spec.CYCLE_T = {**spec.CYCLE_T,
                    mybir.EngineType.Activation: 1e9 / 0.90e9,
                    mybir.EngineType.DVE: 1e9 / 0.83e9}
    spec.SEM_DELAY = 120
    spec.PE_CYCLE = 1e9 / 2.1e9
    spec.DMA_CYCLE = spec.DMA_CYCLE * 1.5


def _build_nc(dbg=False):
    _calibrate_cost_model()
    nc = bacc.Bacc("TRN2", target_bir_lowering=False, debug=False)

    xr = nc.dram_tensor("xr", [NTT, 128, 8, TT], BF16, kind="ExternalInput").ap()
    wqkvr = nc.dram_tensor("wqkvr", [3, 128, 8, CPC], BF16,
                           kind="ExternalInput").ap()
    wtr = nc.dram_tensor("wtr", [128, 8, C], BF16, kind="ExternalInput").ap()
    cosx = nc.dram_tensor("cosx", [128, T], BF16, kind="ExternalInput").ap()
    sinx = nc.dram_tensor("sinx", [128, T], BF16, kind="ExternalInput").ap()
    rt = nc.dram_tensor("rt", [128, 128], BF16, kind="ExternalInput").ap()
    idb = nc.dram_tensor("idb", [128, 128], BF16, kind="ExternalInput").ap()
    mask2 = nc.dram_tensor("mask2", [128, 2, 128], BF16,
                           kind="ExternalInput").ap()
    yblk = nc.dram_tensor("yblk", [4, 2, 128, 512], F32, kind="ExternalOutput").ap()
    if dbg:
        qdbg = nc.dram_tensor("qdbg", [128, T], BF16, kind="ExternalOutput").ap()
        kdbg = nc.dram_tensor("kdbg", [128, T], BF16, kind="ExternalOutput").ap()
        vdbg = nc.dram_tensor("vdbg", [128, 16, Dh + 1], BF16,
                              kind="ExternalOutput").ap()
        adbg = nc.dram_tensor("adbg", [128, 4, TT], BF16,
                              kind="ExternalOutput").ap()
        hdbg = nc.dram_tensor("hdbg", [128, 8, 128], BF16,
                              kind="ExternalOutput").ap()

    with tile.TileContext(nc) as tc, ExitStack() as ctx:
        const = ctx.enter_context(tc.tile_pool(name="const", bufs=1))
        qkpool = ctx.enter_context(tc.tile_pool(name="qk", bufs=2))
        vpool = ctx.enter_context(tc.tile_pool(name="vnat", bufs=4))
        attp = ctx.enter_context(tc.tile_pool(name="attp", bufs=2))
        tmp = ctx.enter_context(tc.tile_pool(name="tmp", bufs=2))
        pbp = ctx.enter_context(tc.tile_pool(name="pbp", bufs=4))
        ahpool = ctx.enter_context(tc.tile_pool(name="ahp", bufs=2))
        psMM = ctx.enter_context(tc.tile_pool(name="psMM", bufs=2, space="PSUM"))
        psQK = ctx.enter_context(tc.tile_pool(name="psQK", bufs=2, space="PSUM"))
        psAC = ctx.enter_context(tc.tile_pool(name="psAC", bufs=2, space="PSUM"))

        # ---- input loads, in order of first use ----
        wqkv_sb = const.tile([128, 3, 8, CPC], BF16)
        nc.sync.dma_start(wqkv_sb[:, 0], wqkvr[0])
        x_sb = []
        for tt in range(NTT):
            x_sb.append(const.tile([128, 8, TT], BF16, name=f"x_sb{tt}"))
        nc.sync.dma_start(x_sb[0][:], xr[0])
        nc.sync.dma_start(wqkv_sb[:, 1], wqkvr[1])
        nc.sync.dma_start(wqkv_sb[:, 2], wqkvr[2])
        rt_sb = const.tile([128, 128], BF16)
        nc.sync.dma_start(rt_sb[:], rt[:])
        id_sb = const.tile([128, 128], BF16)
        nc.sync.dma_start(id_sb[:], idb[:])
        cos_sb = const.tile([128, T], BF16)
        nc.sync.dma_start(cos_sb[:], cosx[:])
        sin_sb = const.tile([128, T], BF16)
        nc.sync.dma_start(sin_sb[:], sinx[:])
        nc.sync.dma_start(x_sb[1][:], xr[1])
        mask_sb = const.tile([128, 2, 128], BF16)
        nc.sync.dma_start(mask_sb[:], mask2[:])
        nc.sync.dma_start(x_sb[2][:], xr[2])
        nc.sync.dma_start(x_sb[3][:], xr[3])
        wt_sb = const.tile([128, 8, C], BF16)
        nc.sync.dma_start(wt_sb[:], wtr[:])

        def stage_a(hp, tt, q_sb, k_sb, v_nat):
            """QKV projection + RoPE + V transpose for one t-tile."""
            ts = slice(tt * TT, (tt + 1) * TT)
            gps = {}
            gb = {}
            rot = {}
            # q/k projections, with RoPE staged behind each
            for grp, gi in (("q", 0), ("k", 1)):
                f0 = hp * 128
                gps[grp] = psMM.tile([128, TT], F32, tag="mm",
                                     name=f"gps_{grp}_{hp}_{tt}")
                for cc in range(8):
                    nc.tensor.matmul(gps[grp][:],
                                     wqkv_sb[:, gi, cc, f0:f0 + 128],
                                     x_sb[tt][:, cc, :],
                                     start=(cc == 0), stop=(cc == 7))
                # PSUM -> SBUF bf16 staging copy on ACT
                gb[grp] = tmp.tile([128, TT], BF16, tag=f"gb{grp}", name=f"gb_{grp}_{hp}_{tt}")
                with tc.high_priority():
                    nc.any.tensor_copy(gb[grp][:], gps[grp][:])
                if grp == "k":
                    # rot-q emitted here so PE has work while gb-k copies
                    rot["q"] = psMM.tile([128, TT], F32, tag="mm",
                                         name=f"rot_q_{hp}_{tt}")
                    nc.tensor.matmul(rot["q"][:], rt_sb[:], gb["q"][:],
                                     start=True, stop=True)
            # RoPE combine for q: dest = gb*cos (DVE 2x) + rot*sin (Pool)
            def rope_combine(grp, dest):
                m1 = tmp.tile([128, TT], BF16, tag="m1", name=f"m1_{grp}_{hp}_{tt}")
                m2 = tmp.tile([128, TT], BF16, tag="m2", name=f"m2_{grp}_{hp}_{tt}")
                with tc.high_priority():
                    nc.gpsimd.tensor_mul(m1[:], gb[grp][:], cos_sb[:, ts])
                    nc.vector.tensor_mul(m2[:], rot[grp][:], sin_sb[:, ts])
                    nc.vector.tensor_add(dest[:, ts], m1[:], m2[:])

            rope_combine("q", q_sb)
            # v projection (PE work covering the q-combine + rot-q release)
            f0 = hp * 128
            gps["v"] = psMM.tile([128, TT], F32, tag="mm",
                                 name=f"gps_v_{hp}_{tt}")
            for cc in range(8):
                nc.tensor.matmul(gps["v"][:], wqkv_sb[:, 2, cc, f0:f0 + 128],
                                 x_sb[tt][:, cc, :],
                                 start=(cc == 0), stop=(cc == 7))
            vf = tmp.tile([128, TT], BF16, tag="vf", name=f"vf_{hp}_{tt}")
            with tc.high_priority():
                nc.vector.tensor_copy(vf[:], gps["v"][:])
            rot["k"] = psMM.tile([128, TT], F32, tag="mm",
                                 name=f"rot_k_{hp}_{tt}")
            nc.tensor.matmul(rot["k"][:], rt_sb[:], gb["k"][:],
                             start=True, stop=True)
            rope_combine("k", k_sb)
            # V transpose: 4x [128,128] bf16 PE transposes into one PSUM tile
            tps = psMM.tile([128, 4, 128], BF16, tag="mm",
                            name=f"tps_{hp}_{tt}")
            for st in range(4):
                nc.tensor.transpose(tps[:, st, :], vf[:, st * 128:(st + 1) * 128],
                                    id_sb[:])
            with tc.high_priority():
                for hl in range(2):
                    nc.vector.tensor_copy(
                        v_nat[hl][:, 4 * tt:4 * tt + 4, 0:Dh],
                        tps[:, :, hl * 64:hl * 64 + 64])

        def stage_b(hp, tt, q_sb, k_sb, v_nat, att_sb):
            """Causal attention for queries in tile tt, pipelined chunks.

            The QK moving operand reads q in (j,k')-major order (tau = j*32+k',
            t = 512*tt + 16*k' + j), free for the PE, so every downstream
            elementwise op is contiguous."""
            ts = slice(tt * TT, (tt + 1) * TT)
            njs = 4 * tt + 4
            acc = [psAC.tile([Dh + 1, TT], F32, tag="acc",
                             name=f"acc_{hp}_{tt}_{hl}") for hl in range(2)]
            pend = []  # (j, pb) entries awaiting AV, lag 2 behind QK
            for j in range(njs):
                sj = slice(j * SC, (j + 1) * SC)
                qk = psQK.tile([128, 2, TT], F32, tag="qk",
                               name=f"qk_{hp}_{tt}_{j}")
                for hl in range(2):
                    hb = hl * 64
                    nc.tensor.matmul(qk[:, hl], k_sb[hb:hb + 64, sj],
                                     q_sb[hb:hb + 64, ts],
                                     start=True, stop=True)
                pb = pbp.tile([128, 2, TT], BF16, tag="pb",
                              name=f"pb_{hp}_{tt}_{j}")
                c = j - 4 * tt
                if c < 0:
                    nc.scalar.activation(pb[:], qk[:], EXP, scale=SCALE)
                else:
                    off = 128 * c
                    if c == 0:
                        nc.scalar.activation(pb[:], qk[:], EXP, scale=SCALE)
                    else:
                        nc.gpsimd.memset(pb[:, :, 0:off], 0.0)
                        nc.scalar.activation(pb[:, :, off:], qk[:, :, off:],
                                             EXP, scale=SCALE)
                    nc.vector.tensor_mul(pb[:, :, off:off + 128],
                                         pb[:, :, off:off + 128],
                                         mask_sb[:])
                if len(pend) >= 2:
                    pj, ppb = pend.pop(0)
                    for hl in range(2):
                        nc.tensor.matmul(acc[hl][:], v_nat[hl][:, pj, :],
                                         ppb[:, hl],
                                         start=(pj == 0), stop=False)
                pend.append((j, pb))
            for pj, ppb in pend:
                for hl in range(2):
                    nc.tensor.matmul(acc[hl][:], v_nat[hl][:, pj, :],
                                     ppb[:, hl],
                                     start=(pj == 0), stop=(pj == njs - 1))
            # normalize into att_sb (layout [d, tt, tau])
            for hl in range(2):
                zrow = tmp.tile([1, TT], F32, tag="zrow",
                                name=f"zrow_{hp}_{tt}_{hl}")
                zi = tmp.tile([1, TT], F32, tag="zi", name=f"zi_{hp}_{tt}_{hl}")
                zb = tmp.tile([64, TT], F32, tag="zb", name=f"zb_{hp}_{tt}_{hl}")
                with tc.high_priority(offset=300):
                    nc.vector.tensor_copy(zrow[:], acc[hl][Dh:Dh + 1, :])
                    nc.vector.reciprocal_approx_fast(out=zi[:], in_=zrow[:])
                    nc.gpsimd.partition_broadcast(zb[:], zi[:], channels=64)
                    nc.vector.tensor_mul(att_sb[hl * 64:hl * 64 + 64, tt, :],
                                         acc[hl][0:Dh, :], zb[:])

        def repack(hp, att_sb, ahts):
            """att [d, tt, (k' j)] (t-linear) -> aht [two*64+d, cc, (tt k')]."""
            for hl in range(2):
                aht = ahpool.tile([128, 8, 128], BF16, tag="aht",
                                  name=f"aht_{hp}_{hl}")
                attv = att_sb[hl * 64:hl * 64 + 64].rearrange(
                    "d tt (k j) -> d j tt k", j=16)
                for two in range(2):
                    for cc in range(8):
                        nc.any.tensor_copy(
                            aht[two * 64:two * 64 + 64, cc, :].rearrange(
                                "d (tt k) -> d tt k", tt=4),
                            attv[:, 2 * cc + two])
                ahts.append(aht)

        def yproj(hp, hl, aht):
            blk = hp * 2 + hl
            for ot in range(2):
                ypss = psMM.tile([128, 512], F32, tag="mm",
                                 name=f"yps_{hp}_{hl}_{ot}")
                for cc in range(8):
                    nc.tensor.matmul(ypss[:], aht[:, cc, :],
                                     wt_sb[:, cc, ot * 512:(ot + 1) * 512],
                                     start=(cc == 0), stop=(cc == 7))
                yo = tmp.tile([128, 512], F32, tag="yo", bufs=4,
                              name=f"yo_{hp}_{hl}_{ot}")
                with tc.high_priority():
                    nc.any.tensor_copy(yo[:], ypss[:])
                nc.sync.dma_start(yblk[blk, ot], yo[:])

        prev = None  # (att_sb, ahts, hp) of previous head pair
        for hp in range(2):
            q_sb = qkpool.tile([128, T], BF16, tag="q", name=f"q_{hp}")
            k_sb = qkpool.tile([128, T], BF16, tag="k", name=f"k_{hp}")
            v_nat = [vpool.tile([128, T // SC, Dh + 1], BF16, tag="vnat",
                                name=f"vnat_{hp}_{hl}") for hl in range(2)]
            for hl in range(2):
                nc.gpsimd.memset(v_nat[hl][:, :, Dh:Dh + 1], 1.0)
            att_sb = attp.tile([128, 4, TT], BF16, tag="att",
                               name=f"att_{hp}")

            for tt in range(NTT):
                stage_a(hp, tt, q_sb, k_sb, v_nat)
            if prev is not None:
                # output projection of previous head pair; the scheduler
                # slots these into this head pair's ACT-bound windows
                patt, pahts, php = prev
                yproj(php, 0, pahts[0])
                yproj(php, 1, pahts[1])
                prev = None
            ahts = []
            for tt in range(NTT):
                stage_b(hp, tt, q_sb, k_sb, v_nat, att_sb)
            repack(hp, att_sb, ahts)
            prev = (att_sb, ahts, hp)
            if dbg and hp == 0:
                nc.sync.dma_start(qdbg[:], q_sb[:])
                nc.sync.dma_start(kdbg[:], k_sb[:])
                nc.sync.dma_start(vdbg[:], v_nat[0][:])
                nc.sync.dma_start(adbg[:], att_sb[:])
                nc.sync.dma_start(hdbg[:], ahts[0][:])

        patt, pahts, php = prev
        yproj(php, 0, pahts[0])
        yproj(php, 1, pahts[1])

    nc.compile()
    return nc


def _get_nc():
    global _compiled_nc
    if _compiled_nc is None:
        _compiled_nc = _build_nc()
    return _compiled_nc


def _host_tables():
    pos = np.arange(T, dtype=np.float32)[:, None]
    inv = np.exp(np.arange(0, Dh, 2, dtype=np.float32)
                 * (-math.log(10000.0) / Dh))
    ang = pos * inv                       # (T, 32)
    sin, cos = np.sin(ang), np.cos(ang)   # (T, 32)
    idx = np.arange(128) % HALF           # d % 32
    cos_ext = np.ascontiguousarray(cos[:, idx].T).astype(NPBF16)  # (128, T)
    sin_ext = np.ascontiguousarray(sin[:, idx].T).astype(NPBF16)

    R = np.zeros((128, 128), dtype=np.float32)
    for blk in (0, 64):
        for m in range(HALF):
            R[blk + m, blk + m + HALF] = -1.0
            R[blk + m + HALF, blk + m] = 1.0
    rt = np.ascontiguousarray(R.T).astype(NPBF16)

    s_i = np.arange(128)[:, None]
    t_i = np.arange(128)[None, :]
    mask01 = (t_i >= s_i).astype(np.float32).astype(NPBF16)
    mask2 = np.ascontiguousarray(
        np.broadcast_to(mask01[:, None, :], (128, 2, 128)))
    ident = np.eye(128, dtype=np.float32).astype(NPBF16)
    return cos_ext, sin_ext, rt, mask2, ident


def kernel(x, w_qkv, w_proj):
    x = np.asarray(x)
    w_qkv = np.asarray(w_qkv)
    w_proj = np.asarray(w_proj)
    nc = _get_nc()
    in_maps = build_in_maps(x, w_qkv, w_proj)
    res = run_bass_kernel_spmd(nc, in_maps, core_ids=list(range(NCORES)))
    y = np.zeros((B, T, C), dtype=np.float32)
    for c in range(NCORES):
        b, g = c // 4, c % 4
        yb = res.results[c]["yblk"]  # [4, 2, 128, 512]
        y[b, 512 * g:512 * g + 512, :] = yb.transpose(0, 2, 1, 3).reshape(512, C)
    return y


def build_in_maps(x, w_qkv, w_proj):
    cos_ext, sin_ext, rt, mask2, ident = _host_tables()
    wq4 = w_qkv.reshape(3, H, Dh, C)
    # w_proj^T packed [p, cc, o]
    wtr = np.ascontiguousarray(
        w_proj.T.reshape(8, 128, C).transpose(1, 0, 2)).astype(NPBF16)
    in_maps = []
    xr_cache = {}
    for c in range(NCORES):
        b, g = c // 4, c % 4
        hs = slice(4 * g, 4 * g + 4)
        wq = wq4[0, hs].reshape(CPC, C)
        wk = wq4[1, hs].reshape(CPC, C)
        wv = wq4[2, hs].reshape(CPC, C)
        # [p, g, cc, f]: wqkvr[p, g, cc, f] = w_g[f, cc*128+p] (g in q,k,v)
        wqkvr = np.ascontiguousarray(
            np.stack([wq, wk, wv], 0).reshape(3, CPC, 8, 128)
            .transpose(0, 3, 2, 1)).astype(NPBF16)
        if b not in xr_cache:
            xT = x[b].T  # (C, T)
            xr_cache[b] = np.ascontiguousarray(
                xT.reshape(8, 128, NTT, TT).transpose(2, 1, 0, 3)).astype(NPBF16)
        in_maps.append({
            "xr": xr_cache[b],
            "wqkvr": wqkvr,
            "wtr": wtr,
            "cosx": cos_ext, "sinx": sin_ext,
            "rt": rt, "idb": ident, "mask2": mask2,
        })
    return in_maps
